# revision 30
# baseline (speedup 1.0000x reference)
"""Trainium2 Bass kernel for nn_Box2FeatureGenerator.

Pipeline per CAV: per-box MLP -> rasterize (last-box-wins scatter) ->
3 residual conv blocks (conv3x3 + per-sample BN + relu).

Sharding: 8 cores = 4 CAVs x 2 H-halves. Each core computes exactly its
128 owned rows; after each conv stage the pair cores exchange their
boundary output rows via a masked pairwise AllReduce (posted right
after the BN-stats AllReduce and consumed by the reordered-last
boundary groups, so its latency is hidden), replacing redundant
halo compute.

All feature maps / conv weights / scatter data are fp16 (PSUM still
accumulates fp32; BN stats and affine params fp32). 16-bit matmuls
stream at ~221 ns per 512-free matmul on HW (the PE weight-load
overlaps; f32r self-loading runs ~286 ns), and all feature DMA traffic
halves vs fp32. fp16 keeps quantization error ~8x below bf16
(measured rel err 1.7e-3 vs 1.1e-2). Conv weights are pre-transposed
and pre-cast on the host, so there is no in-kernel weight staging
pass. BN applies are chunked (6 rows) so the PE restarts sooner after
each stats AllReduce; the stats round-trip DMAs post from the scalar
queue (HWDGE) to skip the SP backlog; the output is written fp16 and
upcast on the host. The residual/final scale-adds are chunked like
the BN applies to shorten the stats-to-restart critical path, and the
final-pass residual reads prefetch 3 groups deep. Measured: 3.65 ms
(5.02 ms f32r baseline), max rel err 1.73e-3.
"""
import sys as _sys
import types as _types
import numpy as np

# antenv.axon_hooks shim: the image's antenv lacks this module; boot
# degrades silently. run_bass_kernel_spmd(trace=True) needs it.
if "antenv.axon_hooks" not in _sys.modules:
    _hm = _types.ModuleType("antenv.axon_hooks")
    _hm._hook = None
    def _set_hook(h):
        _hm._hook = h
    def _get_hook():
        return _hm._hook
    _hm.set_axon_ntff_profile_hook = _set_hook
    _hm.get_axon_ntff_profile_hook = _get_hook
    _sys.modules["antenv.axon_hooks"] = _hm
    try:
        from trn_agent_boot.trn_boot import _ntff_profile_via_ctypes
        _hm.set_axon_ntff_profile_hook(
            _ntff_profile_via_ctypes("/opt/axon/libaxon_pjrt.so"))
    except Exception:
        pass

import concourse.bacc as bacc
import concourse.mybir as mybir
import concourse.tile as tile
from concourse import bass_utils
from concourse.bass import ts
from concourse.masks import make_identity

F32 = mybir.dt.float32
F32R = mybir.dt.float32r
F16 = mybir.dt.float16
FH = np.float16
AF = mybir.ActivationFunctionType
ALU = mybir.AluOpType

# problem constants (hardcoded per spec)
B, N, C, H, W = 4, 32, 256, 256, 256
N_CORES = 8
EPS = 1e-5
HALO = 6
R_BUF = 128 + 2 * HALO          # 140 buffer rows per core
OWN0, OWN1 = HALO, HALO + 128   # owned rows in buffer coords: [6, 134)
G_ROWS = 16                     # rows per conv group
NPIX_STATS = float(H * W)       # BN stats count (full CAV)

_TRACE = False
LAST_EXEC_NS = None
_PROGRAM = None


def _rowpairs(start, end):
    return [(r, min(r + 2, end) - r) for r in range(start, end, 2)]


def _groups(lo, hi):
    gs = []
    if lo < OWN0:
        gs.append((lo, OWN0, False))
    for i in range(128 // G_ROWS):
        gs.append((OWN0 + i * G_ROWS, OWN0 + (i + 1) * G_ROWS, True))
    if hi > OWN1:
        gs.append((OWN1, hi, False))
    return gs


def _build_program():
    nc = bacc.Bacc("TRN2", target_bir_lowering=False, debug=False,
                   num_devices=N_CORES)

    # ---- external inputs (per core) ----
    def inp(name, shape, dt=F32):
        return nc.dram_tensor(name, list(shape), dt, kind="ExternalInput").ap()

    objT_d = inp("objT", (25, N))
    gx0m_d = inp("gx0m", (N, 1))
    gx1_d = inp("gx1", (N, 1))
    gy0m_d = inp("gy0m", (N, 1))
    gy1_d = inp("gy1", (N, 1))
    score_d = inp("score_row", (1, N))
    W1_d = inp("W1", (25, C))
    W2_d = inp("W2c", (128, 2, C))
    W3_d = inp("W3c", (128, 2, C))
    b1_d = inp("b1c", (128, 2))
    b2_d = inp("b2c", (128, 2))
    b3_d = inp("b3c", (128, 2))
    convW_d = inp("convW", (6, 128, 18, C), F16)
    gam_d = inp("gammaT", (128, 12))
    bet_d = inp("betaT", (128, 12))
    iota256_d = inp("iota256", (128, W))
    iotaH_d = inp("iotaH", (128, R_BUF))
    mask_d = inp("mask2d", (128, R_BUF), F16)
    ones_d = inp("onesK1", (1, 128))
    U_d = inp("Utri", (N, N), F16)
    rmask_d = inp("rmaskx", (128, 2, 2, W), F16)

    out_d = nc.dram_tensor("out", [2, 128, 128, W], F16,
                           kind="ExternalOutput").ap()

    with tile.TileContext(nc) as tc:
        with (
            tc.tile_pool(name="const", bufs=1) as cst,
            tc.tile_pool(name="dram", bufs=1, space="DRAM") as dramp,
            tc.tile_pool(name="ccd", bufs=1, space="DRAM") as ccd,
        ):
            # ---- DRAM scratch (fp16 feature maps, buffer-row coords) ----
            def feat(name):
                return dramp.tile([2, 128, R_BUF, W], F16, name=name)

            x0 = feat("x0")
            y1_0, y2_0 = feat("y1_0"), feat("y2_0")
            y1_1, y2_1 = feat("y1_1"), feat("y2_1")
            y1_2, y2_2 = feat("y1_2"), feat("y2_2")
            r1, r2 = feat("r1"), feat("r2")

            # ---- persistent constants ----
            iota256 = cst.tile([128, W], F32)
            nc.sync.dma_start(iota256[:], iota256_d[:])
            iotaH = cst.tile([128, R_BUF], F32)
            nc.sync.dma_start(iotaH[:], iotaH_d[:])
            mask2d = cst.tile([128, R_BUF], F16)
            nc.sync.dma_start(mask2d[:], mask_d[:])
            gam = cst.tile([128, 12], F32)
            nc.sync.dma_start(gam[:], gam_d[:])
            bet = cst.tile([128, 12], F32)
            nc.sync.dma_start(bet[:], bet_d[:])
            zeros = cst.tile([128, 1], F16)
            nc.vector.memset(zeros[:], 0.0)
            epsc = cst.tile([128, 1], F32)
            nc.vector.memset(epsc[:], EPS)
            ident = cst.tile([128, 128], F32)
            make_identity(nc, ident[:])
            s_bn = cst.tile([128, 12], F32)
            t_bn = cst.tile([128, 12], F32)
            Ur = cst.tile([32, 32], F16)
            nc.sync.dma_start(Ur[:], U_d[:])
            rmask = cst.tile([128, 2, 2, W], F16)
            nc.sync.dma_start(rmask[:], rmask_d[:])

            # ---- MLP + bounds + scatter ----
            objf = cst.tile([32, 2, 128], F16)     # scatter lhsT
            coverx = cst.tile([32, W], F16)
            covery = cst.tile([32, R_BUF], F16)

            with (
                tc.tile_pool(name="mlp", bufs=1) as mlp,
                tc.tile_pool(name="mps", bufs=2, space="PSUM") as mps,
            ):
                # ---- box AABB cover masks first (DVE work overlaps PE MLP) ----
                def covermask(lo_m_d, hi_d, iota, cover_out):
                    lo = mlp.tile([32, 1], F32, tag="bnds1",
                                  name=f"lo_{cover_out.tensor.name}")
                    hi = mlp.tile([32, 1], F32, tag="bnds2",
                                  name=f"hi_{cover_out.tensor.name}")
                    nc.sync.dma_start(lo[:], lo_m_d[:])
                    nc.sync.dma_start(hi[:], hi_d[:])
                    m1 = mlp.tile([32, iota.shape[1]], F32, tag="bndm",
                                  name=f"m1_{cover_out.tensor.name}")
                    # bounds are exact small integers -> scalar path exact
                    nc.vector.tensor_scalar(m1[:], iota[:32, :], lo[:], None,
                                            op0=ALU.is_gt)
                    nc.vector.scalar_tensor_tensor(
                        out=cover_out[:], in0=iota[:32, :], scalar=hi[:],
                        in1=m1[:], op0=ALU.is_le, op1=ALU.mult)

                covermask(gx0m_d, gx1_d, iota256, coverx)
                covermask(gy0m_d, gy1_d, iotaH, covery)

                # MLP weights (f32r; tiny fraction of runtime)
                w1_32 = mlp.tile([25, C], F32)
                nc.sync.dma_start(w1_32[:], W1_d[:])
                w1r = mlp.tile([25, C], F32R)
                nc.vector.tensor_copy(w1r[:], w1_32[:])
                w2_32 = mlp.tile([128, 2, C], F32)
                nc.sync.dma_start(w2_32[:], W2_d[:])
                w2r = mlp.tile([128, 2, C], F32R)
                nc.vector.tensor_copy(w2r[:], w2_32[:])
                w3_32 = mlp.tile([128, 2, C], F32)
                nc.sync.dma_start(w3_32[:], W3_d[:])
                w3r = mlp.tile([128, 2, C], F32R)
                nc.vector.tensor_copy(w3r[:], w3_32[:])
                b1 = mlp.tile([128, 2], F32)
                nc.sync.dma_start(b1[:], b1_d[:])
                b2 = mlp.tile([128, 2], F32)
                nc.sync.dma_start(b2[:], b2_d[:])
                b3 = mlp.tile([128, 2], F32)
                nc.sync.dma_start(b3[:], b3_d[:])
                obj32 = mlp.tile([25, N], F32)
                nc.sync.dma_start(obj32[:], objT_d[:])
                objr = mlp.tile([25, N], F32R)
                nc.vector.tensor_copy(objr[:], obj32[:])
                ones32 = mlp.tile([1, 128], F32)
                nc.sync.dma_start(ones32[:], ones_d[:])
                onesr = mlp.tile([1, 128], F32R)
                nc.vector.tensor_copy(onesr[:], ones32[:])
                sc32 = mlp.tile([1, N], F32)
                nc.sync.dma_start(sc32[:], score_d[:])
                scr = mlp.tile([1, N], F32R)
                nc.vector.tensor_copy(scr[:], sc32[:])

                # score broadcast to 128 partitions
                pb = mps.tile([128, N], F32, tag="mp", name="pb")
                nc.tensor.matmul(pb[:], onesr[:], scr[:], start=True, stop=True)
                scoreB = mlp.tile([128, N], F32)
                nc.scalar.copy(scoreB[:], pb[:])

                h1 = mlp.tile([128, 2, N], F32R)
                h2 = mlp.tile([128, 2, N], F32R)
                ofT = mlp.tile([128, 2, N], F32)
                for mc in range(2):
                    p1 = mps.tile([128, N], F32, tag="mp", name=f"p1_{mc}")
                    nc.tensor.matmul(p1[:], w1r[:, ts(mc, 128)], objr[:],
                                     start=True, stop=True)
                    nc.scalar.activation(h1[:, mc, :], p1[:], AF.Relu,
                                         bias=b1[:, mc:mc + 1])
                for mc in range(2):
                    p2 = mps.tile([128, N], F32, tag="mp", name=f"p2_{mc}")
                    for kc in range(2):
                        nc.tensor.matmul(p2[:], w2r[:, kc, ts(mc, 128)],
                                         h1[:, kc, :],
                                         start=(kc == 0), stop=(kc == 1))
                    nc.scalar.activation(h2[:, mc, :], p2[:], AF.Relu,
                                         bias=b2[:, mc:mc + 1])
                for mc in range(2):
                    p3 = mps.tile([128, N], F32, tag="mp", name=f"p3_{mc}")
                    for kc in range(2):
                        nc.tensor.matmul(p3[:], w3r[:, kc, ts(mc, 128)],
                                         h2[:, kc, :],
                                         start=(kc == 0), stop=(kc == 1))
                    # ofT = (h3 + b3) * score
                    nc.vector.scalar_tensor_tensor(
                        out=ofT[:, mc, :], in0=p3[:], scalar=b3[:, mc:mc + 1],
                        in1=scoreB[:], op0=ALU.add, op1=ALU.mult)
                # transpose obj_feat -> [32, 2, 128] bf16
                for mc in range(2):
                    pt = mps.tile([32, 128], F32, tag="mpt", name=f"pt_{mc}",
                                  bufs=2)
                    nc.tensor.transpose(pt[:], ofT[:, mc, :], ident[:])
                    nc.scalar.copy(objf[:, mc, :], pt[:])

            # ---- scatter groups (MLP pools closed; PSUM free) ----
            with (
                tc.tile_pool(name="scat", bufs=2) as scp,
                tc.tile_pool(name="cgps", bufs=4, space="PSUM") as cgps,
                tc.tile_pool(name="scps", bufs=4, space="PSUM") as scps,
            ):
                for gs0 in range(0, R_BUF, G_ROWS):
                    ge0 = min(gs0 + G_ROWS, R_BUF)
                    rows = ge0 - gs0
                    npx = rows * W
                    cov = scp.tile([32, G_ROWS, W], F16, tag="cov",
                                   name=f"cov_{gs0}")
                    nc.vector.tensor_tensor(
                        out=cov[:, :rows, :],
                        in0=covery[:, gs0:ge0].unsqueeze(2)
                            .broadcast_to([32, rows, W]),
                        in1=coverx[:].unsqueeze(1)
                            .broadcast_to([32, rows, W]),
                        op=ALU.mult)
                    cov2 = cov[:, :rows, :].rearrange("n r w -> n (r w)")
                    oh = scp.tile([32, G_ROWS, W], F16, tag="oh",
                                  name=f"oh_{gs0}")
                    oh2 = oh[:, :rows, :].rearrange("n r w -> n (r w)")
                    nsl = npx // 512
                    for sl in range(nsl):
                        cg = cgps.tile([32, 512], F32, tag="cg",
                                       name=f"cg_{gs0}_{sl}")
                        nc.tensor.matmul(cg[:], Ur[:], cov2[:, ts(sl, 512)],
                                         start=True, stop=True)
                        # onehot = (count_gt == 0) * cover
                        nc.vector.scalar_tensor_tensor(
                            out=oh2[:, ts(sl, 512)], in0=cg[:], scalar=0.0,
                            in1=cov2[:, ts(sl, 512)],
                            op0=ALU.is_equal, op1=ALU.mult)
                    for mc in range(2):
                        xsb = scp.tile([128, G_ROWS, W], F16, tag="xsb",
                                       name=f"xsb_{gs0}_{mc}")
                        x2 = xsb[:, :rows, :].rearrange("c r w -> c (r w)")
                        for sl in range(nsl):
                            sp = scps.tile([128, 512], F32, tag="sc",
                                           name=f"sp_{gs0}_{mc}_{sl}")
                            nc.tensor.matmul(sp[:], objf[:, mc, :],
                                             oh2[:, ts(sl, 512)],
                                             start=True, stop=True)
                            nc.scalar.copy(x2[:, ts(sl, 512)], sp[:])
                        nc.sync.dma_start(x0[mc, :, gs0:ge0, :],
                                          xsb[:, :rows, :])

            # ---- conv stages ----
            convs = [
                dict(g=0, src="raw", src_t=x0, out_t=y1_0, lo=1, hi=139),
                dict(g=1, src="bn", src_t=y1_0, sg=0, out_t=y2_0, lo=2, hi=138),
                dict(g=2, src="res", src_t=y2_0, res_t=x0, sg=1, out_t=y1_1,
                     r_out=r1, lo=3, hi=137),
                dict(g=3, src="bn", src_t=y1_1, sg=2, out_t=y2_1, lo=4, hi=136),
                dict(g=4, src="res", src_t=y2_1, res_t=r1, sg=3, out_t=y1_2,
                     r_out=r2, lo=5, hi=135),
                dict(g=5, src="bn", src_t=y1_2, sg=4, out_t=y2_2, lo=6, hi=134),
            ]

            with (
                tc.tile_pool(name="cw", bufs=2) as cwp,
                tc.tile_pool(name="cin", bufs=2) as cinp,
                tc.tile_pool(name="crt", bufs=3) as crtp,
                tc.tile_pool(name="cy", bufs=6) as cyp,
                tc.tile_pool(name="cst2", bufs=1) as cst2,
                tc.tile_pool(name="cps", bufs=8, space="PSUM") as cps,
            ):
                sqs = cst2.tile([128, 512], F32, name="sqs", bufs=2)
                sq16 = cst2.tile([128, 512], F16, name="sq16", bufs=2)
                for cv in convs:
                    g = cv["g"]
                    wr = cwp.tile([128, 18, C], F16, tag="wr", name=f"cwr_{g}")
                    nc.sync.dma_start(wr[:], convW_d[g])
                    st_sum = [cst2.tile([128, 64], F32, name=f"ssum_{g}_{m}",
                                        tag=f"ssum{m}") for m in range(2)]
                    st_sq = [cst2.tile([128, 64], F32, name=f"ssq_{g}_{m}",
                                       tag=f"ssq{m}") for m in range(2)]
                    glist = _groups(OWN0, OWN1)
                    # boundary groups last: their input halo row arrives via
                    # the cross-core row exchange of the previous stage
                    proc = glist[1:7] + [glist[0], glist[7]]
                    owned_idx = 0
                    for (start, end, owned) in proc:
                        cnt = end - start + 2
                        in_t = []
                        for kc in range(2):
                            it = cinp.tile([128, 18, W + 2], F16,
                                           tag=f"in{kc}",
                                           name=f"in_{g}_{start}_{kc}")
                            in_t.append(it)
                            sub = it[:, :cnt, 1:W + 1]
                            nc.sync.dma_start(
                                sub, cv["src_t"][kc, :, start - 1:end + 1, :])
                            # zero pad columns
                            nc.vector.tensor_copy(
                                it[:, :cnt, 0:1],
                                zeros[:].unsqueeze(1).broadcast_to([128, cnt, 1]))
                            nc.vector.tensor_copy(
                                it[:, :cnt, W + 1:W + 2],
                                zeros[:].unsqueeze(1).broadcast_to([128, cnt, 1]))
                            if cv["src"] == "bn":
                                col = kc * 6 + cv["sg"]
                                # chunked: PE starts on the first rows sooner
                                # at stage boundaries
                                for c0 in range(0, cnt, 6):
                                    c1 = min(c0 + 6, cnt)
                                    nc.scalar.activation(
                                        sub[:, c0:c1, :], sub[:, c0:c1, :],
                                        AF.Relu,
                                        bias=t_bn[:, col:col + 1],
                                        scale=s_bn[:, col:col + 1])
                            elif cv["src"] == "res":
                                col = kc * 6 + cv["sg"]
                                rt = crtp.tile([128, 18, W], F16, tag="rt",
                                               name=f"rt_{g}_{start}_{kc}")
                                nc.sync.dma_start(
                                    rt[:, :cnt, :],
                                    cv["res_t"][kc, :, start - 1:end + 1, :])
                                for c0 in range(0, cnt, 6):
                                    c1 = min(c0 + 6, cnt)
                                    nc.vector.scalar_tensor_tensor(
                                        out=sub[:, c0:c1, :],
                                        in0=sub[:, c0:c1, :],
                                        scalar=s_bn[:, col:col + 1],
                                        in1=rt[:, c0:c1, :],
                                        op0=ALU.mult, op1=ALU.add)
                                for c0 in range(0, cnt, 6):
                                    c1 = min(c0 + 6, cnt)
                                    nc.scalar.activation(
                                        sub[:, c0:c1, :], sub[:, c0:c1, :],
                                        AF.Relu,
                                        bias=t_bn[:, col:col + 1])
                            if cv["src"] != "raw":
                                # zero out image-invalid halo rows
                                if start < 7:
                                    k = min(7 - start, cnt)
                                    nc.vector.tensor_tensor(
                                        out=it[:, :k, 1:W + 1],
                                        in0=it[:, :k, 1:W + 1],
                                        in1=mask2d[:, start - 1:start - 1 + k]
                                            .unsqueeze(2)
                                            .broadcast_to([128, k, W]),
                                        op=ALU.mult)
                                if end > OWN1 - 1:
                                    k0 = (OWN1 - (start - 1))
                                    k = cnt - k0
                                    nc.vector.tensor_tensor(
                                        out=it[:, k0:cnt, 1:W + 1],
                                        in0=it[:, k0:cnt, 1:W + 1],
                                        in1=mask2d[:, start - 1 + k0:end + 1]
                                            .unsqueeze(2)
                                            .broadcast_to([128, k, W]),
                                        op=ALU.mult)
                            if cv["src"] == "res":
                                # write r_next back (skip 2-row overlap with
                                # the next group, except for the last group)
                                wlo, whi = start - 1, end + 1
                                if (start, end) != (glist[-1][0], glist[-1][1]):
                                    whi = end - 1
                                nc.sync.dma_start(
                                    cv["r_out"][kc, :, wlo:whi, :],
                                    it[:, :whi - wlo, 1:W + 1])
                        pairs = _rowpairs(start, end)
                        for mc in range(2):
                            psums = [cps.tile([128, 512], F32, tag="cp",
                                              name=f"ps_{g}_{start}_{mc}_{i}")
                                     for i in range(len(pairs))]
                            for t9 in range(9):
                                dy, dx = t9 // 3, t9 % 3
                                for kc in range(2):
                                    lhsT = wr[:, t9 * 2 + kc, ts(mc, 128)]
                                    for i, (pr, prn) in enumerate(pairs):
                                        loc = pr - (start - 1)
                                        rhs = in_t[kc][:, loc + dy - 1:
                                                       loc + dy - 1 + prn,
                                                       dx:dx + W]
                                        nc.tensor.matmul(
                                            psums[i][:, :prn * W], lhsT, rhs,
                                            start=(t9 == 0 and kc == 0),
                                            stop=(t9 == 8 and kc == 1))
                            for i, (pr, prn) in enumerate(pairs):
                                ysb = cyp.tile([128, 2, W], F16, tag="ysb",
                                               name=f"ysb_{g}_{start}_{mc}_{i}")
                                y2d = ysb[:, :prn, :].rearrange(
                                    "c r w -> c (r w)")
                                pv = psums[i][:, :prn * W]
                                if owned:
                                    idx = owned_idx + i
                                    nc.scalar.activation(
                                        y2d, pv, AF.Copy,
                                        accum_out=st_sum[mc][:, idx:idx + 1])
                                    # square-stats on DVE from the evicted
                                    # fp16 tile: the ACT queue drains ~7us
                                    # sooner at stage end (stats critical path)
                                    nc.vector.tensor_tensor(
                                        out=sq16[:, :prn * W], in0=y2d,
                                        in1=y2d, op=ALU.mult)
                                    nc.vector.tensor_reduce(
                                        st_sq[mc][:, idx:idx + 1],
                                        sq16[:, :prn * W],
                                        axis=mybir.AxisListType.X, op=ALU.add)
                                else:
                                    nc.scalar.copy(y2d, pv)
                                nc.sync.dma_start(
                                    cv["out_t"][mc, :, pr:pr + prn, :],
                                    ysb[:, :prn, :])
                        if owned:
                            owned_idx += len(pairs)

                    # ---- BN stats: reduce, AllReduce pair, compute s/t ----
                    pay = cst2.tile([128, 4], F32, name=f"pay_{g}", tag="pay",
                                    bufs=2)
                    for m in range(2):
                        nc.vector.tensor_reduce(pay[:, 2 * m:2 * m + 1],
                                                st_sum[m][:],
                                                axis=mybir.AxisListType.X,
                                                op=ALU.add)
                        nc.vector.tensor_reduce(pay[:, 2 * m + 1:2 * m + 2],
                                                st_sq[m][:],
                                                axis=mybir.AxisListType.X,
                                                op=ALU.add)
                    sin = cst2.tile([128, 4], F32, name=f"sin_{g}", tag="sin",
                                    bufs=2)
                    cc_in = ccd.tile([128, 4], F32, name=f"ccin_{g}")
                    cc_out = ccd.tile([128, 4], F32, name=f"ccout_{g}")
                    # post the stats round-trip DMAs from the scalar queue
                    # (HWDGE): the SP queue is backlogged with feature-map
                    # traffic and would add ~10us to this critical chain
                    nc.scalar.dma_start(cc_in[:], pay[:])
                    nc.gpsimd.collective_compute(
                        "AllReduce", ALU.add,
                        replica_groups=[[0, 1], [2, 3], [4, 5], [6, 7]],
                        ins=[cc_in.opt()], outs=[cc_out.opt()])
                    nc.scalar.dma_start(sin[:], cc_out[:])
                    for m in range(2):
                        col = m * 6 + g
                        mean = cst2.tile([128, 1], F32, name=f"mean_{g}_{m}",
                                         tag="bnw0", bufs=2)
                        em2 = cst2.tile([128, 1], F32, name=f"em2_{g}_{m}",
                                        tag="bnw1", bufs=2)
                        nc.vector.tensor_scalar_mul(mean[:],
                                                    sin[:, 2 * m:2 * m + 1],
                                                    1.0 / NPIX_STATS)
                        nc.vector.tensor_scalar_mul(em2[:],
                                                    sin[:, 2 * m + 1:2 * m + 2],
                                                    1.0 / NPIX_STATS)
                        var = cst2.tile([128, 1], F32, name=f"var_{g}_{m}",
                                        tag="bnw2", bufs=2)
                        nc.vector.tensor_tensor(out=var[:], in0=mean[:],
                                                in1=mean[:], op=ALU.mult)
                        nc.vector.tensor_sub(var[:], em2[:], var[:])
                        sd = cst2.tile([128, 1], F32, name=f"sd_{g}_{m}",
                                       tag="bnw3", bufs=2)
                        nc.scalar.activation(sd[:], var[:], AF.Sqrt, bias=epsc[:])
                        inv = cst2.tile([128, 1], F32, name=f"inv_{g}_{m}",
                                        tag="bnw4", bufs=2)
                        nc.vector.reciprocal(inv[:], sd[:])
                        nc.vector.tensor_tensor(out=s_bn[:, col:col + 1],
                                                in0=gam[:, col:col + 1],
                                                in1=inv[:], op=ALU.mult)
                        tmp = cst2.tile([128, 1], F32, name=f"tmp_{g}_{m}",
                                        tag="bnw5", bufs=2)
                        nc.vector.tensor_tensor(out=tmp[:], in0=mean[:],
                                                in1=s_bn[:, col:col + 1],
                                                op=ALU.mult)
                        nc.vector.tensor_sub(t_bn[:, col:col + 1],
                                             bet[:, col:col + 1], tmp[:])

                    # ---- cross-core halo row exchange. Pairwise AllReduce
                    # (same proven path as the BN stats): each core
                    # contributes its first/last owned output row in its own
                    # slot (partner slot zeroed via the host-built rmask), so
                    # the sum is a concatenation. slot0 = even core's row 133
                    # (odd's buffer row 5); slot1 = odd core's row 6 (even's
                    # row 134). The image-invalid side of each pair is zeroed
                    # by mask2d at consumption.
                    if g < 5:
                        ot = cv["out_t"]
                        rbs = cst2.tile([128, 2, 2, W], F16, name=f"rbs_{g}",
                                        tag="rbs", bufs=2)
                        for kc in range(2):
                            nc.scalar.dma_start(rbs[:, kc, 0, :],
                                                ot[kc, :, OWN1 - 1, :])
                            nc.scalar.dma_start(rbs[:, kc, 1, :],
                                                ot[kc, :, OWN0, :])
                        nc.vector.tensor_tensor(out=rbs[:], in0=rbs[:],
                                                in1=rmask[:], op=ALU.mult)
                        cc_ri = ccd.tile([128, 2, 2, W], F16, name=f"ccri_{g}")
                        cc_ro = ccd.tile([128, 2, 2, W], F16, name=f"ccro_{g}")
                        nc.scalar.dma_start(cc_ri[:], rbs[:])
                        nc.gpsimd.collective_compute(
                            "AllReduce", ALU.add,
                            replica_groups=[[0, 1], [2, 3], [4, 5], [6, 7]],
                            ins=[cc_ri.opt()], outs=[cc_ro.opt()])
                        rbr = cst2.tile([128, 2, 2, W], F16, name=f"rbr_{g}",
                                        tag="rbr", bufs=2)
                        nc.scalar.dma_start(rbr[:], cc_ro[:])
                        for kc in range(2):
                            nc.scalar.dma_start(ot[kc, :, OWN0 - 1, :],
                                                rbr[:, kc, 0, :])
                            nc.scalar.dma_start(ot[kc, :, OWN1, :],
                                                rbr[:, kc, 1, :])

                # ---- final: out = relu(bn(y2_2) + r2), owned rows ----
                with tc.tile_pool(name="fin", bufs=2) as finp:
                    for i in range(128 // G_ROWS):
                        gs0 = OWN0 + i * G_ROWS
                        ge0 = gs0 + G_ROWS
                        for kc in range(2):
                            col = kc * 6 + 5
                            ft = finp.tile([128, G_ROWS, W], F16,
                                           tag=f"fy{kc}", name=f"ft_{i}_{kc}")
                            rt = finp.tile([128, G_ROWS, W], F16,
                                           tag=f"fr{kc}", name=f"frt_{i}_{kc}",
                                           bufs=3)
                            nc.sync.dma_start(ft[:], y2_2[kc, :, gs0:ge0, :])
                            nc.sync.dma_start(rt[:], r2[kc, :, gs0:ge0, :])
                            for c0 in range(0, G_ROWS, 8):
                                nc.vector.scalar_tensor_tensor(
                                    out=ft[:, c0:c0 + 8, :],
                                    in0=ft[:, c0:c0 + 8, :],
                                    scalar=s_bn[:, col:col + 1],
                                    in1=rt[:, c0:c0 + 8, :],
                                    op0=ALU.mult, op1=ALU.add)
                            for q in range(0, G_ROWS, 4):
                                osb = finp.tile([128, 4, W], F16, tag="osb",
                                                name=f"osb_{i}_{kc}_{q}",
                                                bufs=4)
                                nc.scalar.activation(osb[:], ft[:, q:q + 4, :],
                                                     AF.Relu,
                                                     bias=t_bn[:, col:col + 1])
                                nc.sync.dma_start(
                                    out_d[kc, :, gs0 - OWN0 + q:
                                          gs0 - OWN0 + q + 4, :],
                                    osb[:])

    nc.compile()
    return nc


def _host_inputs(pred_box, pred_score, W1, b1, W2, b2, W3, b3, conv_w,
                 gamma, beta):
    """Build the 8 per-core input maps."""
    f = np.float32
    # conv weights: [blk, j, co, ci, ky, kx] -> [g, ci128, (ky kx kc), co]
    cw = conv_w.reshape(6, 256, 2, 128, 3, 3)
    cw = np.ascontiguousarray(cw.transpose(0, 3, 4, 5, 2, 1))
    convW = cw.reshape(6, 128, 18, 256).astype(FH)
    gamT = np.ascontiguousarray(
        gamma.reshape(6, 2, 128).transpose(1, 2, 0)).reshape(2, 128, 6)
    betT = np.ascontiguousarray(
        beta.reshape(6, 2, 128).transpose(1, 2, 0)).reshape(2, 128, 6)
    gamT = np.concatenate([gamT[0], gamT[1]], axis=1)  # [128, 12]
    betT = np.concatenate([betT[0], betT[1]], axis=1)
    W2c = np.ascontiguousarray(W2.reshape(2, 128, 256).transpose(1, 0, 2))
    W3c = np.ascontiguousarray(W3.reshape(2, 128, 256).transpose(1, 0, 2))
    iota256 = np.broadcast_to(np.arange(W, dtype=f), (128, W)).copy()
    # count_gt[n] = sum_m lhsT[m, n] * cover[m]; lhsT[m, n] = 1 iff m > n
    U = (np.arange(N)[:, None] > np.arange(N)[None, :]).astype(FH)
    ones1 = np.ones((1, 128), f)

    in_maps = []
    for c in range(N_CORES):
        b = c // 2
        s = 128 * (c % 2)
        geom = pred_box[b].reshape(N, 24).astype(f)
        objT = np.concatenate([geom.T, pred_score[b][None, :].astype(f)], 0)
        cx = pred_box[b, :, :, 0].astype(f)
        cy = pred_box[b, :, :, 1].astype(f)
        gx = np.floor((cx + f(51.2)) / f(0.4))
        gy = np.floor((cy + f(51.2)) / f(0.4))
        gx0 = np.clip(gx.min(-1), 0, W - 1).astype(f)
        gx1v = np.clip(gx.max(-1), 0, W - 1).astype(f)
        gy0 = np.clip(gy.min(-1), 0, H - 1).astype(f)
        gy1v = np.clip(gy.max(-1), 0, H - 1).astype(f)
        rows = np.arange(s - HALO, s + 128 + HALO, dtype=np.int64)
        iotaH = np.broadcast_to(rows.astype(f), (128, R_BUF)).copy()
        mask2d = np.broadcast_to(
            ((rows >= 0) & (rows < H)).astype(FH), (128, R_BUF)).copy()
        rmaskx = np.zeros((128, 2, 2, W), FH)
        rmaskx[:, :, c % 2, :] = 1
        in_maps.append({
            "objT": np.ascontiguousarray(objT),
            "rmaskx": rmaskx,
            "gx0m": (gx0 - 1)[:, None], "gx1": gx1v[:, None],
            "gy0m": (gy0 - 1)[:, None], "gy1": gy1v[:, None],
            "score_row": pred_score[b][None, :].astype(f),
            "W1": W1.astype(f), "W2c": W2c.astype(f), "W3c": W3c.astype(f),
            "b1c": b1.reshape(2, 128).T.astype(f).copy(),
            "b2c": b2.reshape(2, 128).T.astype(f).copy(),
            "b3c": b3.reshape(2, 128).T.astype(f).copy(),
            "convW": convW,
            "gammaT": gamT.astype(f), "betaT": betT.astype(f),
            "iota256": iota256, "iotaH": iotaH, "mask2d": mask2d,
            "onesK1": ones1, "Utri": U,
        })
    return in_maps


def kernel(**inputs):
    global _PROGRAM, LAST_EXEC_NS
    if _PROGRAM is None:
        _PROGRAM = _build_program()
    nc = _PROGRAM
    in_maps = _host_inputs(**{k: np.asarray(v) for k, v in inputs.items()})
    res = bass_utils.run_bass_kernel_spmd(
        nc, in_maps, core_ids=list(range(N_CORES)), trace=_TRACE)
    LAST_EXEC_NS = res.exec_time_ns
    full = np.empty((B, C, H, W), np.float32)
    for c in range(N_CORES):
        b = c // 2
        s = 128 * (c % 2)
        o = res.results[c]["out"]
        full[b, 0:128, s:s + 128, :] = o[0]
        full[b, 128:256, s:s + 128, :] = o[1]
    return full


# revision 32
# speedup vs baseline: 1.0061x; 1.0061x over previous
"""Trainium2 Bass kernel for nn_Box2FeatureGenerator.

Pipeline per CAV: per-box MLP -> rasterize (last-box-wins scatter) ->
3 residual conv blocks (conv3x3 + per-sample BN + relu).

Sharding: 8 cores = 4 CAVs x 2 H-halves. Each core computes exactly its
128 owned rows; after each conv stage the pair cores exchange their
boundary output rows via a masked pairwise AllReduce (posted right
after the BN-stats AllReduce and consumed by the reordered-last
boundary groups, so its latency is hidden), replacing redundant
halo compute.

All feature maps / conv weights / scatter data are fp16 (PSUM still
accumulates fp32; BN stats and affine params fp32). 16-bit matmuls
stream at ~221 ns per 512-free matmul on HW (the PE weight-load
overlaps; f32r self-loading runs ~286 ns), and all feature DMA traffic
halves vs fp32. fp16 keeps quantization error ~8x below bf16
(measured rel err 1.7e-3 vs 1.1e-2). Conv weights are pre-transposed
and pre-cast on the host, so there is no in-kernel weight staging
pass. BN applies are chunked (6 rows) so the PE restarts sooner after
each stats AllReduce; the stats round-trip DMAs post from the scalar
queue (HWDGE) to skip the SP backlog; the output is written fp16 and
upcast on the host. The residual/final scale-adds are chunked like
the BN applies to shorten the stats-to-restart critical path, and the
final-pass residual reads prefetch 3 groups deep. Measured: 3.65 ms
(5.02 ms f32r baseline), max rel err 1.73e-3.
"""
import sys as _sys
import types as _types
import numpy as np

# antenv.axon_hooks shim: the image's antenv lacks this module; boot
# degrades silently. run_bass_kernel_spmd(trace=True) needs it.
if "antenv.axon_hooks" not in _sys.modules:
    _hm = _types.ModuleType("antenv.axon_hooks")
    _hm._hook = None
    def _set_hook(h):
        _hm._hook = h
    def _get_hook():
        return _hm._hook
    _hm.set_axon_ntff_profile_hook = _set_hook
    _hm.get_axon_ntff_profile_hook = _get_hook
    _sys.modules["antenv.axon_hooks"] = _hm
    try:
        from trn_agent_boot.trn_boot import _ntff_profile_via_ctypes
        _hm.set_axon_ntff_profile_hook(
            _ntff_profile_via_ctypes("/opt/axon/libaxon_pjrt.so"))
    except Exception:
        pass

import concourse.bacc as bacc
import concourse.mybir as mybir
import concourse.tile as tile
from concourse import bass_utils
from concourse.bass import ts
from concourse.masks import make_identity

F32 = mybir.dt.float32
F32R = mybir.dt.float32r
F16 = mybir.dt.float16
FH = np.float16
AF = mybir.ActivationFunctionType
ALU = mybir.AluOpType

# problem constants (hardcoded per spec)
B, N, C, H, W = 4, 32, 256, 256, 256
N_CORES = 8
EPS = 1e-5
HALO = 6
R_BUF = 128 + 2 * HALO          # 140 buffer rows per core
OWN0, OWN1 = HALO, HALO + 128   # owned rows in buffer coords: [6, 134)
G_ROWS = 16                     # rows per conv group
NPIX_STATS = float(H * W)       # BN stats count (full CAV)

_TRACE = False
LAST_EXEC_NS = None
_PROGRAM = None


def _rowpairs(start, end):
    return [(r, min(r + 2, end) - r) for r in range(start, end, 2)]


def _groups(lo, hi):
    gs = []
    if lo < OWN0:
        gs.append((lo, OWN0, False))
    for i in range(128 // G_ROWS):
        gs.append((OWN0 + i * G_ROWS, OWN0 + (i + 1) * G_ROWS, True))
    if hi > OWN1:
        gs.append((OWN1, hi, False))
    return gs


def _build_program():
    nc = bacc.Bacc("TRN2", target_bir_lowering=False, debug=False,
                   num_devices=N_CORES)

    # ---- external inputs (per core) ----
    def inp(name, shape, dt=F32):
        return nc.dram_tensor(name, list(shape), dt, kind="ExternalInput").ap()

    objT_d = inp("objT", (25, N))
    gx0m_d = inp("gx0m", (N, 1))
    gx1_d = inp("gx1", (N, 1))
    gy0m_d = inp("gy0m", (N, 1))
    gy1_d = inp("gy1", (N, 1))
    score_d = inp("score_row", (1, N))
    W1_d = inp("W1", (25, C))
    W2_d = inp("W2c", (128, 2, C))
    W3_d = inp("W3c", (128, 2, C))
    b1_d = inp("b1c", (128, 2))
    b2_d = inp("b2c", (128, 2))
    b3_d = inp("b3c", (128, 2))
    convW_d = inp("convW", (6, 128, 18, C), F16)
    gam_d = inp("gammaT", (128, 12))
    bet_d = inp("betaT", (128, 12))
    iota256_d = inp("iota256", (128, W))
    iotaH_d = inp("iotaH", (128, R_BUF))
    mask_d = inp("mask2d", (128, R_BUF), F16)
    ones_d = inp("onesK1", (1, 128))
    U_d = inp("Utri", (N, N), F16)
    rmask_d = inp("rmaskx", (128, 2, 2, W), F16)

    out_d = nc.dram_tensor("out", [2, 128, 128, W], F16,
                           kind="ExternalOutput").ap()

    with tile.TileContext(nc) as tc:
        with (
            tc.tile_pool(name="const", bufs=1) as cst,
            tc.tile_pool(name="dram", bufs=1, space="DRAM") as dramp,
            tc.tile_pool(name="ccd", bufs=1, space="DRAM") as ccd,
        ):
            # ---- DRAM scratch (fp16 feature maps, buffer-row coords) ----
            def feat(name):
                return dramp.tile([2, 128, R_BUF, W], F16, name=name)

            x0 = feat("x0")
            y1_0, y2_0 = feat("y1_0"), feat("y2_0")
            y1_1, y2_1 = feat("y1_1"), feat("y2_1")
            y1_2, y2_2 = feat("y1_2"), feat("y2_2")
            r1, r2 = feat("r1"), feat("r2")

            # ---- persistent constants ----
            iota256 = cst.tile([128, W], F32)
            nc.sync.dma_start(iota256[:], iota256_d[:])
            iotaH = cst.tile([128, R_BUF], F32)
            nc.sync.dma_start(iotaH[:], iotaH_d[:])
            mask2d = cst.tile([128, R_BUF], F16)
            nc.sync.dma_start(mask2d[:], mask_d[:])
            gam = cst.tile([128, 12], F32)
            nc.sync.dma_start(gam[:], gam_d[:])
            bet = cst.tile([128, 12], F32)
            nc.sync.dma_start(bet[:], bet_d[:])
            zeros = cst.tile([128, 1], F16)
            nc.vector.memset(zeros[:], 0.0)
            epsc = cst.tile([128, 1], F32)
            nc.vector.memset(epsc[:], EPS)
            ident = cst.tile([128, 128], F32)
            make_identity(nc, ident[:])
            s_bn = cst.tile([128, 12], F32)
            t_bn = cst.tile([128, 12], F32)
            Ur = cst.tile([32, 32], F16)
            nc.sync.dma_start(Ur[:], U_d[:])
            rmask = cst.tile([128, 2, 2, W], F16)
            nc.sync.dma_start(rmask[:], rmask_d[:])

            # ---- MLP + bounds + scatter ----
            objf = cst.tile([32, 2, 128], F16)     # scatter lhsT
            coverx = cst.tile([32, W], F16)
            covery = cst.tile([32, R_BUF], F16)

            with (
                tc.tile_pool(name="mlp", bufs=1) as mlp,
                tc.tile_pool(name="mps", bufs=2, space="PSUM") as mps,
            ):
                # ---- box AABB cover masks first (DVE work overlaps PE MLP) ----
                def covermask(lo_m_d, hi_d, iota, cover_out):
                    lo = mlp.tile([32, 1], F32, tag="bnds1",
                                  name=f"lo_{cover_out.tensor.name}")
                    hi = mlp.tile([32, 1], F32, tag="bnds2",
                                  name=f"hi_{cover_out.tensor.name}")
                    nc.sync.dma_start(lo[:], lo_m_d[:])
                    nc.sync.dma_start(hi[:], hi_d[:])
                    m1 = mlp.tile([32, iota.shape[1]], F32, tag="bndm",
                                  name=f"m1_{cover_out.tensor.name}")
                    # bounds are exact small integers -> scalar path exact
                    nc.vector.tensor_scalar(m1[:], iota[:32, :], lo[:], None,
                                            op0=ALU.is_gt)
                    nc.vector.scalar_tensor_tensor(
                        out=cover_out[:], in0=iota[:32, :], scalar=hi[:],
                        in1=m1[:], op0=ALU.is_le, op1=ALU.mult)

                covermask(gx0m_d, gx1_d, iota256, coverx)
                covermask(gy0m_d, gy1_d, iotaH, covery)

                # MLP weights (f32r; tiny fraction of runtime)
                w1_32 = mlp.tile([25, C], F32)
                nc.sync.dma_start(w1_32[:], W1_d[:])
                w1r = mlp.tile([25, C], F32R)
                nc.vector.tensor_copy(w1r[:], w1_32[:])
                w2_32 = mlp.tile([128, 2, C], F32)
                nc.sync.dma_start(w2_32[:], W2_d[:])
                w2r = mlp.tile([128, 2, C], F32R)
                nc.vector.tensor_copy(w2r[:], w2_32[:])
                w3_32 = mlp.tile([128, 2, C], F32)
                nc.sync.dma_start(w3_32[:], W3_d[:])
                w3r = mlp.tile([128, 2, C], F32R)
                nc.vector.tensor_copy(w3r[:], w3_32[:])
                b1 = mlp.tile([128, 2], F32)
                nc.sync.dma_start(b1[:], b1_d[:])
                b2 = mlp.tile([128, 2], F32)
                nc.sync.dma_start(b2[:], b2_d[:])
                b3 = mlp.tile([128, 2], F32)
                nc.sync.dma_start(b3[:], b3_d[:])
                obj32 = mlp.tile([25, N], F32)
                nc.sync.dma_start(obj32[:], objT_d[:])
                objr = mlp.tile([25, N], F32R)
                nc.vector.tensor_copy(objr[:], obj32[:])
                ones32 = mlp.tile([1, 128], F32)
                nc.sync.dma_start(ones32[:], ones_d[:])
                onesr = mlp.tile([1, 128], F32R)
                nc.vector.tensor_copy(onesr[:], ones32[:])
                sc32 = mlp.tile([1, N], F32)
                nc.sync.dma_start(sc32[:], score_d[:])
                scr = mlp.tile([1, N], F32R)
                nc.vector.tensor_copy(scr[:], sc32[:])

                # score broadcast to 128 partitions
                pb = mps.tile([128, N], F32, tag="mp", name="pb")
                nc.tensor.matmul(pb[:], onesr[:], scr[:], start=True, stop=True)
                scoreB = mlp.tile([128, N], F32)
                nc.scalar.copy(scoreB[:], pb[:])

                h1 = mlp.tile([128, 2, N], F32R)
                h2 = mlp.tile([128, 2, N], F32R)
                ofT = mlp.tile([128, 2, N], F32)
                for mc in range(2):
                    p1 = mps.tile([128, N], F32, tag="mp", name=f"p1_{mc}")
                    nc.tensor.matmul(p1[:], w1r[:, ts(mc, 128)], objr[:],
                                     start=True, stop=True)
                    nc.scalar.activation(h1[:, mc, :], p1[:], AF.Relu,
                                         bias=b1[:, mc:mc + 1])
                for mc in range(2):
                    p2 = mps.tile([128, N], F32, tag="mp", name=f"p2_{mc}")
                    for kc in range(2):
                        nc.tensor.matmul(p2[:], w2r[:, kc, ts(mc, 128)],
                                         h1[:, kc, :],
                                         start=(kc == 0), stop=(kc == 1))
                    nc.scalar.activation(h2[:, mc, :], p2[:], AF.Relu,
                                         bias=b2[:, mc:mc + 1])
                for mc in range(2):
                    p3 = mps.tile([128, N], F32, tag="mp", name=f"p3_{mc}")
                    for kc in range(2):
                        nc.tensor.matmul(p3[:], w3r[:, kc, ts(mc, 128)],
                                         h2[:, kc, :],
                                         start=(kc == 0), stop=(kc == 1))
                    # ofT = (h3 + b3) * score
                    nc.vector.scalar_tensor_tensor(
                        out=ofT[:, mc, :], in0=p3[:], scalar=b3[:, mc:mc + 1],
                        in1=scoreB[:], op0=ALU.add, op1=ALU.mult)
                # transpose obj_feat -> [32, 2, 128] bf16
                for mc in range(2):
                    pt = mps.tile([32, 128], F32, tag="mpt", name=f"pt_{mc}",
                                  bufs=2)
                    nc.tensor.transpose(pt[:], ofT[:, mc, :], ident[:])
                    nc.scalar.copy(objf[:, mc, :], pt[:])

            # ---- scatter groups (MLP pools closed; PSUM free) ----
            with (
                tc.tile_pool(name="scat", bufs=2) as scp,
                tc.tile_pool(name="cgps", bufs=4, space="PSUM") as cgps,
                tc.tile_pool(name="scps", bufs=4, space="PSUM") as scps,
            ):
                for gs0 in range(0, R_BUF, G_ROWS):
                    ge0 = min(gs0 + G_ROWS, R_BUF)
                    rows = ge0 - gs0
                    npx = rows * W
                    cov = scp.tile([32, G_ROWS, W], F16, tag="cov",
                                   name=f"cov_{gs0}")
                    nc.vector.tensor_tensor(
                        out=cov[:, :rows, :],
                        in0=covery[:, gs0:ge0].unsqueeze(2)
                            .broadcast_to([32, rows, W]),
                        in1=coverx[:].unsqueeze(1)
                            .broadcast_to([32, rows, W]),
                        op=ALU.mult)
                    cov2 = cov[:, :rows, :].rearrange("n r w -> n (r w)")
                    oh = scp.tile([32, G_ROWS, W], F16, tag="oh",
                                  name=f"oh_{gs0}")
                    oh2 = oh[:, :rows, :].rearrange("n r w -> n (r w)")
                    nsl = npx // 512
                    for sl in range(nsl):
                        cg = cgps.tile([32, 512], F32, tag="cg",
                                       name=f"cg_{gs0}_{sl}")
                        nc.tensor.matmul(cg[:], Ur[:], cov2[:, ts(sl, 512)],
                                         start=True, stop=True)
                        # onehot = (count_gt == 0) * cover
                        nc.vector.scalar_tensor_tensor(
                            out=oh2[:, ts(sl, 512)], in0=cg[:], scalar=0.0,
                            in1=cov2[:, ts(sl, 512)],
                            op0=ALU.is_equal, op1=ALU.mult)
                    for mc in range(2):
                        xsb = scp.tile([128, G_ROWS, W], F16, tag="xsb",
                                       name=f"xsb_{gs0}_{mc}")
                        x2 = xsb[:, :rows, :].rearrange("c r w -> c (r w)")
                        for sl in range(nsl):
                            sp = scps.tile([128, 512], F32, tag="sc",
                                           name=f"sp_{gs0}_{mc}_{sl}")
                            nc.tensor.matmul(sp[:], objf[:, mc, :],
                                             oh2[:, ts(sl, 512)],
                                             start=True, stop=True)
                            nc.scalar.copy(x2[:, ts(sl, 512)], sp[:])
                        nc.sync.dma_start(x0[mc, :, gs0:ge0, :],
                                          xsb[:, :rows, :])

            # ---- conv stages ----
            convs = [
                dict(g=0, src="raw", src_t=x0, out_t=y1_0, lo=1, hi=139),
                dict(g=1, src="bn", src_t=y1_0, sg=0, out_t=y2_0, lo=2, hi=138),
                dict(g=2, src="res", src_t=y2_0, res_t=x0, sg=1, out_t=y1_1,
                     r_out=r1, lo=3, hi=137),
                dict(g=3, src="bn", src_t=y1_1, sg=2, out_t=y2_1, lo=4, hi=136),
                dict(g=4, src="res", src_t=y2_1, res_t=r1, sg=3, out_t=y1_2,
                     r_out=r2, lo=5, hi=135),
                dict(g=5, src="bn", src_t=y1_2, sg=4, out_t=y2_2, lo=6, hi=134),
            ]

            with (
                tc.tile_pool(name="cw", bufs=2) as cwp,
                tc.tile_pool(name="cin", bufs=2) as cinp,
                tc.tile_pool(name="crt", bufs=3) as crtp,
                tc.tile_pool(name="cy", bufs=6) as cyp,
                tc.tile_pool(name="cst2", bufs=1) as cst2,
                tc.tile_pool(name="cps", bufs=8, space="PSUM") as cps,
            ):
                sqs = cst2.tile([128, 512], F32, name="sqs", bufs=2)
                for cv in convs:
                    g = cv["g"]
                    wr = cwp.tile([128, 18, C], F16, tag="wr", name=f"cwr_{g}")
                    nc.sync.dma_start(wr[:], convW_d[g])
                    st_sum = [cst2.tile([128, 64], F32, name=f"ssum_{g}_{m}",
                                        tag=f"ssum{m}") for m in range(2)]
                    st_sq = [cst2.tile([128, 64], F32, name=f"ssq_{g}_{m}",
                                       tag=f"ssq{m}") for m in range(2)]
                    glist = _groups(OWN0, OWN1)
                    # boundary groups last: their input halo row arrives via
                    # the cross-core row exchange of the previous stage
                    proc = glist[1:7] + [glist[0], glist[7]]
                    owned_idx = 0
                    for (start, end, owned) in proc:
                        cnt = end - start + 2
                        in_t = []
                        for kc in range(2):
                            it = cinp.tile([128, 18, W + 2], F16,
                                           tag=f"in{kc}",
                                           name=f"in_{g}_{start}_{kc}")
                            in_t.append(it)
                            sub = it[:, :cnt, 1:W + 1]
                            nc.sync.dma_start(
                                sub, cv["src_t"][kc, :, start - 1:end + 1, :])
                            # zero pad columns
                            nc.vector.tensor_copy(
                                it[:, :cnt, 0:1],
                                zeros[:].unsqueeze(1).broadcast_to([128, cnt, 1]))
                            nc.vector.tensor_copy(
                                it[:, :cnt, W + 1:W + 2],
                                zeros[:].unsqueeze(1).broadcast_to([128, cnt, 1]))
                            if cv["src"] == "bn":
                                col = kc * 6 + cv["sg"]
                                # chunked: PE starts on the first rows sooner
                                # at stage boundaries
                                for c0 in range(0, cnt, 6):
                                    c1 = min(c0 + 6, cnt)
                                    nc.scalar.activation(
                                        sub[:, c0:c1, :], sub[:, c0:c1, :],
                                        AF.Relu,
                                        bias=t_bn[:, col:col + 1],
                                        scale=s_bn[:, col:col + 1])
                            elif cv["src"] == "res":
                                col = kc * 6 + cv["sg"]
                                rt = crtp.tile([128, 18, W], F16, tag="rt",
                                               name=f"rt_{g}_{start}_{kc}")
                                nc.sync.dma_start(
                                    rt[:, :cnt, :],
                                    cv["res_t"][kc, :, start - 1:end + 1, :])
                                for c0 in range(0, cnt, 6):
                                    c1 = min(c0 + 6, cnt)
                                    nc.vector.scalar_tensor_tensor(
                                        out=sub[:, c0:c1, :],
                                        in0=sub[:, c0:c1, :],
                                        scalar=s_bn[:, col:col + 1],
                                        in1=rt[:, c0:c1, :],
                                        op0=ALU.mult, op1=ALU.add)
                                for c0 in range(0, cnt, 6):
                                    c1 = min(c0 + 6, cnt)
                                    nc.scalar.activation(
                                        sub[:, c0:c1, :], sub[:, c0:c1, :],
                                        AF.Relu,
                                        bias=t_bn[:, col:col + 1])
                            if cv["src"] != "raw":
                                # zero out image-invalid halo rows
                                if start < 7:
                                    k = min(7 - start, cnt)
                                    nc.vector.tensor_tensor(
                                        out=it[:, :k, 1:W + 1],
                                        in0=it[:, :k, 1:W + 1],
                                        in1=mask2d[:, start - 1:start - 1 + k]
                                            .unsqueeze(2)
                                            .broadcast_to([128, k, W]),
                                        op=ALU.mult)
                                if end > OWN1 - 1:
                                    k0 = (OWN1 - (start - 1))
                                    k = cnt - k0
                                    nc.vector.tensor_tensor(
                                        out=it[:, k0:cnt, 1:W + 1],
                                        in0=it[:, k0:cnt, 1:W + 1],
                                        in1=mask2d[:, start - 1 + k0:end + 1]
                                            .unsqueeze(2)
                                            .broadcast_to([128, k, W]),
                                        op=ALU.mult)
                            if cv["src"] == "res":
                                # write r_next back (skip 2-row overlap with
                                # the next group, except for the last group)
                                wlo, whi = start - 1, end + 1
                                if (start, end) != (glist[-1][0], glist[-1][1]):
                                    whi = end - 1
                                nc.sync.dma_start(
                                    cv["r_out"][kc, :, wlo:whi, :],
                                    it[:, :whi - wlo, 1:W + 1])
                        pairs = _rowpairs(start, end)
                        for mc in range(2):
                            psums = [cps.tile([128, 512], F32, tag="cp",
                                              name=f"ps_{g}_{start}_{mc}_{i}")
                                     for i in range(len(pairs))]
                            for t9 in range(9):
                                dy, dx = t9 // 3, t9 % 3
                                for kc in range(2):
                                    lhsT = wr[:, t9 * 2 + kc, ts(mc, 128)]
                                    for i, (pr, prn) in enumerate(pairs):
                                        loc = pr - (start - 1)
                                        rhs = in_t[kc][:, loc + dy - 1:
                                                       loc + dy - 1 + prn,
                                                       dx:dx + W]
                                        nc.tensor.matmul(
                                            psums[i][:, :prn * W], lhsT, rhs,
                                            start=(t9 == 0 and kc == 0),
                                            stop=(t9 == 8 and kc == 1))
                            for i, (pr, prn) in enumerate(pairs):
                                ysb = cyp.tile([128, 2, W], F16, tag="ysb",
                                               name=f"ysb_{g}_{start}_{mc}_{i}")
                                y2d = ysb[:, :prn, :].rearrange(
                                    "c r w -> c (r w)")
                                pv = psums[i][:, :prn * W]
                                if owned:
                                    idx = owned_idx + i
                                    nc.scalar.activation(
                                        y2d, pv, AF.Copy,
                                        accum_out=st_sum[mc][:, idx:idx + 1])
                                    nc.scalar.activation(
                                        sqs[:, :prn * W], pv, AF.Square,
                                        accum_out=st_sq[mc][:, idx:idx + 1])
                                else:
                                    nc.scalar.copy(y2d, pv)
                                nc.sync.dma_start(
                                    cv["out_t"][mc, :, pr:pr + prn, :],
                                    ysb[:, :prn, :])
                        if owned:
                            owned_idx += len(pairs)

                    # ---- BN stats: reduce, AllReduce pair, compute s/t ----
                    pay = cst2.tile([128, 4], F32, name=f"pay_{g}", tag="pay",
                                    bufs=2)
                    for m in range(2):
                        nc.vector.tensor_reduce(pay[:, 2 * m:2 * m + 1],
                                                st_sum[m][:],
                                                axis=mybir.AxisListType.X,
                                                op=ALU.add)
                        nc.vector.tensor_reduce(pay[:, 2 * m + 1:2 * m + 2],
                                                st_sq[m][:],
                                                axis=mybir.AxisListType.X,
                                                op=ALU.add)
                    sin = cst2.tile([128, 4], F32, name=f"sin_{g}", tag="sin",
                                    bufs=2)
                    cc_in = ccd.tile([128, 4], F32, name=f"ccin_{g}")
                    cc_out = ccd.tile([128, 4], F32, name=f"ccout_{g}")
                    # post the stats round-trip DMAs from the scalar queue
                    # (HWDGE): the SP queue is backlogged with feature-map
                    # traffic and would add ~10us to this critical chain
                    nc.scalar.dma_start(cc_in[:], pay[:])
                    nc.gpsimd.collective_compute(
                        "AllReduce", ALU.add,
                        replica_groups=[[0, 1], [2, 3], [4, 5], [6, 7]],
                        ins=[cc_in.opt()], outs=[cc_out.opt()])
                    nc.scalar.dma_start(sin[:], cc_out[:])
                    for m in range(2):
                        col = m * 6 + g
                        mean = cst2.tile([128, 1], F32, name=f"mean_{g}_{m}",
                                         tag="bnw0", bufs=2)
                        em2 = cst2.tile([128, 1], F32, name=f"em2_{g}_{m}",
                                        tag="bnw1", bufs=2)
                        nc.vector.tensor_scalar_mul(mean[:],
                                                    sin[:, 2 * m:2 * m + 1],
                                                    1.0 / NPIX_STATS)
                        nc.vector.tensor_scalar_mul(em2[:],
                                                    sin[:, 2 * m + 1:2 * m + 2],
                                                    1.0 / NPIX_STATS)
                        var = cst2.tile([128, 1], F32, name=f"var_{g}_{m}",
                                        tag="bnw2", bufs=2)
                        nc.vector.tensor_tensor(out=var[:], in0=mean[:],
                                                in1=mean[:], op=ALU.mult)
                        nc.vector.tensor_sub(var[:], em2[:], var[:])
                        sd = cst2.tile([128, 1], F32, name=f"sd_{g}_{m}",
                                       tag="bnw3", bufs=2)
                        nc.scalar.activation(sd[:], var[:], AF.Sqrt, bias=epsc[:])
                        inv = cst2.tile([128, 1], F32, name=f"inv_{g}_{m}",
                                        tag="bnw4", bufs=2)
                        nc.vector.reciprocal(inv[:], sd[:])
                        nc.vector.tensor_tensor(out=s_bn[:, col:col + 1],
                                                in0=gam[:, col:col + 1],
                                                in1=inv[:], op=ALU.mult)
                        tmp = cst2.tile([128, 1], F32, name=f"tmp_{g}_{m}",
                                        tag="bnw5", bufs=2)
                        nc.vector.tensor_tensor(out=tmp[:], in0=mean[:],
                                                in1=s_bn[:, col:col + 1],
                                                op=ALU.mult)
                        nc.vector.tensor_sub(t_bn[:, col:col + 1],
                                             bet[:, col:col + 1], tmp[:])

                    # ---- cross-core halo row exchange. Pairwise AllReduce
                    # (same proven path as the BN stats): each core
                    # contributes its first/last owned output row in its own
                    # slot (partner slot zeroed via the host-built rmask), so
                    # the sum is a concatenation. slot0 = even core's row 133
                    # (odd's buffer row 5); slot1 = odd core's row 6 (even's
                    # row 134). The image-invalid side of each pair is zeroed
                    # by mask2d at consumption.
                    if g < 5:
                        ot = cv["out_t"]
                        rbs = cst2.tile([128, 2, 2, W], F16, name=f"rbs_{g}",
                                        tag="rbs", bufs=2)
                        for kc in range(2):
                            nc.gpsimd.dma_start(rbs[:, kc, 0, :],
                                                ot[kc, :, OWN1 - 1, :])
                            nc.gpsimd.dma_start(rbs[:, kc, 1, :],
                                                ot[kc, :, OWN0, :])
                        nc.vector.tensor_tensor(out=rbs[:], in0=rbs[:],
                                                in1=rmask[:], op=ALU.mult)
                        cc_ri = ccd.tile([128, 2, 2, W], F16, name=f"ccri_{g}")
                        cc_ro = ccd.tile([128, 2, 2, W], F16, name=f"ccro_{g}")
                        nc.gpsimd.dma_start(cc_ri[:], rbs[:])
                        nc.gpsimd.collective_compute(
                            "AllReduce", ALU.add,
                            replica_groups=[[0, 1], [2, 3], [4, 5], [6, 7]],
                            ins=[cc_ri.opt()], outs=[cc_ro.opt()])
                        rbr = cst2.tile([128, 2, 2, W], F16, name=f"rbr_{g}",
                                        tag="rbr", bufs=2)
                        nc.gpsimd.dma_start(rbr[:], cc_ro[:])
                        for kc in range(2):
                            nc.gpsimd.dma_start(ot[kc, :, OWN0 - 1, :],
                                                rbr[:, kc, 0, :])
                            nc.gpsimd.dma_start(ot[kc, :, OWN1, :],
                                                rbr[:, kc, 1, :])

                # ---- final: out = relu(bn(y2_2) + r2), owned rows ----
                with tc.tile_pool(name="fin", bufs=2) as finp:
                    for i in range(128 // G_ROWS):
                        gs0 = OWN0 + i * G_ROWS
                        ge0 = gs0 + G_ROWS
                        for kc in range(2):
                            col = kc * 6 + 5
                            ft = finp.tile([128, G_ROWS, W], F16,
                                           tag=f"fy{kc}", name=f"ft_{i}_{kc}")
                            rt = finp.tile([128, G_ROWS, W], F16,
                                           tag=f"fr{kc}", name=f"frt_{i}_{kc}",
                                           bufs=3)
                            nc.sync.dma_start(ft[:], y2_2[kc, :, gs0:ge0, :])
                            nc.sync.dma_start(rt[:], r2[kc, :, gs0:ge0, :])
                            for c0 in range(0, G_ROWS, 8):
                                nc.vector.scalar_tensor_tensor(
                                    out=ft[:, c0:c0 + 8, :],
                                    in0=ft[:, c0:c0 + 8, :],
                                    scalar=s_bn[:, col:col + 1],
                                    in1=rt[:, c0:c0 + 8, :],
                                    op0=ALU.mult, op1=ALU.add)
                            for q in range(0, G_ROWS, 4):
                                osb = finp.tile([128, 4, W], F16, tag="osb",
                                                name=f"osb_{i}_{kc}_{q}",
                                                bufs=4)
                                nc.scalar.activation(osb[:], ft[:, q:q + 4, :],
                                                     AF.Relu,
                                                     bias=t_bn[:, col:col + 1])
                                nc.sync.dma_start(
                                    out_d[kc, :, gs0 - OWN0 + q:
                                          gs0 - OWN0 + q + 4, :],
                                    osb[:])

    nc.compile()
    return nc


def _host_inputs(pred_box, pred_score, W1, b1, W2, b2, W3, b3, conv_w,
                 gamma, beta):
    """Build the 8 per-core input maps."""
    f = np.float32
    # conv weights: [blk, j, co, ci, ky, kx] -> [g, ci128, (ky kx kc), co]
    cw = conv_w.reshape(6, 256, 2, 128, 3, 3)
    cw = np.ascontiguousarray(cw.transpose(0, 3, 4, 5, 2, 1))
    convW = cw.reshape(6, 128, 18, 256).astype(FH)
    gamT = np.ascontiguousarray(
        gamma.reshape(6, 2, 128).transpose(1, 2, 0)).reshape(2, 128, 6)
    betT = np.ascontiguousarray(
        beta.reshape(6, 2, 128).transpose(1, 2, 0)).reshape(2, 128, 6)
    gamT = np.concatenate([gamT[0], gamT[1]], axis=1)  # [128, 12]
    betT = np.concatenate([betT[0], betT[1]], axis=1)
    W2c = np.ascontiguousarray(W2.reshape(2, 128, 256).transpose(1, 0, 2))
    W3c = np.ascontiguousarray(W3.reshape(2, 128, 256).transpose(1, 0, 2))
    iota256 = np.broadcast_to(np.arange(W, dtype=f), (128, W)).copy()
    # count_gt[n] = sum_m lhsT[m, n] * cover[m]; lhsT[m, n] = 1 iff m > n
    U = (np.arange(N)[:, None] > np.arange(N)[None, :]).astype(FH)
    ones1 = np.ones((1, 128), f)

    in_maps = []
    for c in range(N_CORES):
        b = c // 2
        s = 128 * (c % 2)
        geom = pred_box[b].reshape(N, 24).astype(f)
        objT = np.concatenate([geom.T, pred_score[b][None, :].astype(f)], 0)
        cx = pred_box[b, :, :, 0].astype(f)
        cy = pred_box[b, :, :, 1].astype(f)
        gx = np.floor((cx + f(51.2)) / f(0.4))
        gy = np.floor((cy + f(51.2)) / f(0.4))
        gx0 = np.clip(gx.min(-1), 0, W - 1).astype(f)
        gx1v = np.clip(gx.max(-1), 0, W - 1).astype(f)
        gy0 = np.clip(gy.min(-1), 0, H - 1).astype(f)
        gy1v = np.clip(gy.max(-1), 0, H - 1).astype(f)
        rows = np.arange(s - HALO, s + 128 + HALO, dtype=np.int64)
        iotaH = np.broadcast_to(rows.astype(f), (128, R_BUF)).copy()
        mask2d = np.broadcast_to(
            ((rows >= 0) & (rows < H)).astype(FH), (128, R_BUF)).copy()
        rmaskx = np.zeros((128, 2, 2, W), FH)
        rmaskx[:, :, c % 2, :] = 1
        in_maps.append({
            "objT": np.ascontiguousarray(objT),
            "rmaskx": rmaskx,
            "gx0m": (gx0 - 1)[:, None], "gx1": gx1v[:, None],
            "gy0m": (gy0 - 1)[:, None], "gy1": gy1v[:, None],
            "score_row": pred_score[b][None, :].astype(f),
            "W1": W1.astype(f), "W2c": W2c.astype(f), "W3c": W3c.astype(f),
            "b1c": b1.reshape(2, 128).T.astype(f).copy(),
            "b2c": b2.reshape(2, 128).T.astype(f).copy(),
            "b3c": b3.reshape(2, 128).T.astype(f).copy(),
            "convW": convW,
            "gammaT": gamT.astype(f), "betaT": betT.astype(f),
            "iota256": iota256, "iotaH": iotaH, "mask2d": mask2d,
            "onesK1": ones1, "Utri": U,
        })
    return in_maps


def kernel(**inputs):
    global _PROGRAM, LAST_EXEC_NS
    if _PROGRAM is None:
        _PROGRAM = _build_program()
    nc = _PROGRAM
    in_maps = _host_inputs(**{k: np.asarray(v) for k, v in inputs.items()})
    res = bass_utils.run_bass_kernel_spmd(
        nc, in_maps, core_ids=list(range(N_CORES)), trace=_TRACE)
    LAST_EXEC_NS = res.exec_time_ns
    full = np.empty((B, C, H, W), np.float32)
    for c in range(N_CORES):
        b = c // 2
        s = 128 * (c % 2)
        o = res.results[c]["out"]
        full[b, 0:128, s:s + 128, :] = o[0]
        full[b, 128:256, s:s + 128, :] = o[1]
    return full


# revision 33
# speedup vs baseline: 1.0068x; 1.0007x over previous
"""Trainium2 Bass kernel for nn_Box2FeatureGenerator.

Pipeline per CAV: per-box MLP -> rasterize (last-box-wins scatter) ->
3 residual conv blocks (conv3x3 + per-sample BN + relu).

Sharding: 8 cores = 4 CAVs x 2 H-halves. Each core computes exactly its
128 owned rows; after each conv stage the pair cores exchange their
boundary output rows via a masked pairwise AllReduce (posted right
after the BN-stats AllReduce and consumed by the reordered-last
boundary groups, so its latency is hidden), replacing redundant
halo compute.

All feature maps / conv weights / scatter data are fp16 (PSUM still
accumulates fp32; BN stats and affine params fp32). 16-bit matmuls
stream at ~221 ns per 512-free matmul on HW (the PE weight-load
overlaps; f32r self-loading runs ~286 ns), and all feature DMA traffic
halves vs fp32. fp16 keeps quantization error ~8x below bf16
(measured rel err 1.7e-3 vs 1.1e-2). Conv weights are pre-transposed
and pre-cast on the host, so there is no in-kernel weight staging
pass. BN applies are chunked (6 rows) so the PE restarts sooner after
each stats AllReduce; the stats round-trip DMAs post from the scalar
queue (HWDGE) to skip the SP backlog; the output is written fp16 and
upcast on the host. The residual/final scale-adds are chunked like
the BN applies to shorten the stats-to-restart critical path, and the
final-pass residual reads prefetch 3 groups deep. Measured: 3.65 ms
(5.02 ms f32r baseline), max rel err 1.73e-3.

Final trace budget (per core): PE active 3.149 ms (~96% stream eff),
ACT 1.73 ms, SP 1.03 ms, DVE 0.38 ms; PE idle 485 us = 175 us
HBM-bound final tail + 5 BN-stats boundaries of ~34-49 us (eviction
drain + ~9 us collective floor) + 15 us head. Next candidates, in
order: 2D Winograd (~2.25x fewer MACs; elementwise transforms must
spread over DVE/ACT/Pool), SBUF-persisting the last conv output for
the final pass (needs ~65 KB that the pools don't currently spare).
Known-bad: InstTensorTensorReduce wedges the device; DVE-side square
stats regress (Vector queue becomes the stage-end constraint).
"""
import sys as _sys
import types as _types
import numpy as np

# antenv.axon_hooks shim: the image's antenv lacks this module; boot
# degrades silently. run_bass_kernel_spmd(trace=True) needs it.
if "antenv.axon_hooks" not in _sys.modules:
    _hm = _types.ModuleType("antenv.axon_hooks")
    _hm._hook = None
    def _set_hook(h):
        _hm._hook = h
    def _get_hook():
        return _hm._hook
    _hm.set_axon_ntff_profile_hook = _set_hook
    _hm.get_axon_ntff_profile_hook = _get_hook
    _sys.modules["antenv.axon_hooks"] = _hm
    try:
        from trn_agent_boot.trn_boot import _ntff_profile_via_ctypes
        _hm.set_axon_ntff_profile_hook(
            _ntff_profile_via_ctypes("/opt/axon/libaxon_pjrt.so"))
    except Exception:
        pass

import concourse.bacc as bacc
import concourse.mybir as mybir
import concourse.tile as tile
from concourse import bass_utils
from concourse.bass import ts
from concourse.masks import make_identity

F32 = mybir.dt.float32
F32R = mybir.dt.float32r
F16 = mybir.dt.float16
FH = np.float16
AF = mybir.ActivationFunctionType
ALU = mybir.AluOpType

# problem constants (hardcoded per spec)
B, N, C, H, W = 4, 32, 256, 256, 256
N_CORES = 8
EPS = 1e-5
HALO = 6
R_BUF = 128 + 2 * HALO          # 140 buffer rows per core
OWN0, OWN1 = HALO, HALO + 128   # owned rows in buffer coords: [6, 134)
G_ROWS = 16                     # rows per conv group
NPIX_STATS = float(H * W)       # BN stats count (full CAV)

_TRACE = False
LAST_EXEC_NS = None
_PROGRAM = None


def _rowpairs(start, end):
    return [(r, min(r + 2, end) - r) for r in range(start, end, 2)]


def _groups(lo, hi):
    gs = []
    if lo < OWN0:
        gs.append((lo, OWN0, False))
    for i in range(128 // G_ROWS):
        gs.append((OWN0 + i * G_ROWS, OWN0 + (i + 1) * G_ROWS, True))
    if hi > OWN1:
        gs.append((OWN1, hi, False))
    return gs


def _build_program():
    nc = bacc.Bacc("TRN2", target_bir_lowering=False, debug=False,
                   num_devices=N_CORES)

    # ---- external inputs (per core) ----
    def inp(name, shape, dt=F32):
        return nc.dram_tensor(name, list(shape), dt, kind="ExternalInput").ap()

    objT_d = inp("objT", (25, N))
    gx0m_d = inp("gx0m", (N, 1))
    gx1_d = inp("gx1", (N, 1))
    gy0m_d = inp("gy0m", (N, 1))
    gy1_d = inp("gy1", (N, 1))
    score_d = inp("score_row", (1, N))
    W1_d = inp("W1", (25, C))
    W2_d = inp("W2c", (128, 2, C))
    W3_d = inp("W3c", (128, 2, C))
    b1_d = inp("b1c", (128, 2))
    b2_d = inp("b2c", (128, 2))
    b3_d = inp("b3c", (128, 2))
    convW_d = inp("convW", (6, 128, 18, C), F16)
    gam_d = inp("gammaT", (128, 12))
    bet_d = inp("betaT", (128, 12))
    iota256_d = inp("iota256", (128, W))
    iotaH_d = inp("iotaH", (128, R_BUF))
    mask_d = inp("mask2d", (128, R_BUF), F16)
    ones_d = inp("onesK1", (1, 128))
    U_d = inp("Utri", (N, N), F16)
    rmask_d = inp("rmaskx", (128, 2, 2, W), F16)

    out_d = nc.dram_tensor("out", [2, 128, 128, W], F16,
                           kind="ExternalOutput").ap()

    with tile.TileContext(nc) as tc:
        with (
            tc.tile_pool(name="const", bufs=1) as cst,
            tc.tile_pool(name="dram", bufs=1, space="DRAM") as dramp,
            tc.tile_pool(name="ccd", bufs=1, space="DRAM") as ccd,
        ):
            # ---- DRAM scratch (fp16 feature maps, buffer-row coords) ----
            def feat(name):
                return dramp.tile([2, 128, R_BUF, W], F16, name=name)

            x0 = feat("x0")
            y1_0, y2_0 = feat("y1_0"), feat("y2_0")
            y1_1, y2_1 = feat("y1_1"), feat("y2_1")
            y1_2, y2_2 = feat("y1_2"), feat("y2_2")
            r1, r2 = feat("r1"), feat("r2")

            # ---- persistent constants ----
            iota256 = cst.tile([128, W], F32)
            nc.sync.dma_start(iota256[:], iota256_d[:])
            iotaH = cst.tile([128, R_BUF], F32)
            nc.sync.dma_start(iotaH[:], iotaH_d[:])
            mask2d = cst.tile([128, R_BUF], F16)
            nc.sync.dma_start(mask2d[:], mask_d[:])
            gam = cst.tile([128, 12], F32)
            nc.sync.dma_start(gam[:], gam_d[:])
            bet = cst.tile([128, 12], F32)
            nc.sync.dma_start(bet[:], bet_d[:])
            zeros = cst.tile([128, 1], F16)
            nc.vector.memset(zeros[:], 0.0)
            epsc = cst.tile([128, 1], F32)
            nc.vector.memset(epsc[:], EPS)
            ident = cst.tile([128, 128], F32)
            make_identity(nc, ident[:])
            s_bn = cst.tile([128, 12], F32)
            t_bn = cst.tile([128, 12], F32)
            Ur = cst.tile([32, 32], F16)
            nc.sync.dma_start(Ur[:], U_d[:])
            rmask = cst.tile([128, 2, 2, W], F16)
            nc.sync.dma_start(rmask[:], rmask_d[:])

            # ---- MLP + bounds + scatter ----
            objf = cst.tile([32, 2, 128], F16)     # scatter lhsT
            coverx = cst.tile([32, W], F16)
            covery = cst.tile([32, R_BUF], F16)

            with (
                tc.tile_pool(name="mlp", bufs=1) as mlp,
                tc.tile_pool(name="mps", bufs=2, space="PSUM") as mps,
            ):
                # ---- box AABB cover masks first (DVE work overlaps PE MLP) ----
                def covermask(lo_m_d, hi_d, iota, cover_out):
                    lo = mlp.tile([32, 1], F32, tag="bnds1",
                                  name=f"lo_{cover_out.tensor.name}")
                    hi = mlp.tile([32, 1], F32, tag="bnds2",
                                  name=f"hi_{cover_out.tensor.name}")
                    nc.sync.dma_start(lo[:], lo_m_d[:])
                    nc.sync.dma_start(hi[:], hi_d[:])
                    m1 = mlp.tile([32, iota.shape[1]], F32, tag="bndm",
                                  name=f"m1_{cover_out.tensor.name}")
                    # bounds are exact small integers -> scalar path exact
                    nc.vector.tensor_scalar(m1[:], iota[:32, :], lo[:], None,
                                            op0=ALU.is_gt)
                    nc.vector.scalar_tensor_tensor(
                        out=cover_out[:], in0=iota[:32, :], scalar=hi[:],
                        in1=m1[:], op0=ALU.is_le, op1=ALU.mult)

                covermask(gx0m_d, gx1_d, iota256, coverx)
                covermask(gy0m_d, gy1_d, iotaH, covery)

                # MLP weights (f32r; tiny fraction of runtime)
                w1_32 = mlp.tile([25, C], F32)
                nc.sync.dma_start(w1_32[:], W1_d[:])
                w1r = mlp.tile([25, C], F32R)
                nc.vector.tensor_copy(w1r[:], w1_32[:])
                w2_32 = mlp.tile([128, 2, C], F32)
                nc.sync.dma_start(w2_32[:], W2_d[:])
                w2r = mlp.tile([128, 2, C], F32R)
                nc.vector.tensor_copy(w2r[:], w2_32[:])
                w3_32 = mlp.tile([128, 2, C], F32)
                nc.sync.dma_start(w3_32[:], W3_d[:])
                w3r = mlp.tile([128, 2, C], F32R)
                nc.vector.tensor_copy(w3r[:], w3_32[:])
                b1 = mlp.tile([128, 2], F32)
                nc.sync.dma_start(b1[:], b1_d[:])
                b2 = mlp.tile([128, 2], F32)
                nc.sync.dma_start(b2[:], b2_d[:])
                b3 = mlp.tile([128, 2], F32)
                nc.sync.dma_start(b3[:], b3_d[:])
                obj32 = mlp.tile([25, N], F32)
                nc.sync.dma_start(obj32[:], objT_d[:])
                objr = mlp.tile([25, N], F32R)
                nc.vector.tensor_copy(objr[:], obj32[:])
                ones32 = mlp.tile([1, 128], F32)
                nc.sync.dma_start(ones32[:], ones_d[:])
                onesr = mlp.tile([1, 128], F32R)
                nc.vector.tensor_copy(onesr[:], ones32[:])
                sc32 = mlp.tile([1, N], F32)
                nc.sync.dma_start(sc32[:], score_d[:])
                scr = mlp.tile([1, N], F32R)
                nc.vector.tensor_copy(scr[:], sc32[:])

                # score broadcast to 128 partitions
                pb = mps.tile([128, N], F32, tag="mp", name="pb")
                nc.tensor.matmul(pb[:], onesr[:], scr[:], start=True, stop=True)
                scoreB = mlp.tile([128, N], F32)
                nc.scalar.copy(scoreB[:], pb[:])

                h1 = mlp.tile([128, 2, N], F32R)
                h2 = mlp.tile([128, 2, N], F32R)
                ofT = mlp.tile([128, 2, N], F32)
                for mc in range(2):
                    p1 = mps.tile([128, N], F32, tag="mp", name=f"p1_{mc}")
                    nc.tensor.matmul(p1[:], w1r[:, ts(mc, 128)], objr[:],
                                     start=True, stop=True)
                    nc.scalar.activation(h1[:, mc, :], p1[:], AF.Relu,
                                         bias=b1[:, mc:mc + 1])
                for mc in range(2):
                    p2 = mps.tile([128, N], F32, tag="mp", name=f"p2_{mc}")
                    for kc in range(2):
                        nc.tensor.matmul(p2[:], w2r[:, kc, ts(mc, 128)],
                                         h1[:, kc, :],
                                         start=(kc == 0), stop=(kc == 1))
                    nc.scalar.activation(h2[:, mc, :], p2[:], AF.Relu,
                                         bias=b2[:, mc:mc + 1])
                for mc in range(2):
                    p3 = mps.tile([128, N], F32, tag="mp", name=f"p3_{mc}")
                    for kc in range(2):
                        nc.tensor.matmul(p3[:], w3r[:, kc, ts(mc, 128)],
                                         h2[:, kc, :],
                                         start=(kc == 0), stop=(kc == 1))
                    # ofT = (h3 + b3) * score
                    nc.vector.scalar_tensor_tensor(
                        out=ofT[:, mc, :], in0=p3[:], scalar=b3[:, mc:mc + 1],
                        in1=scoreB[:], op0=ALU.add, op1=ALU.mult)
                # transpose obj_feat -> [32, 2, 128] bf16
                for mc in range(2):
                    pt = mps.tile([32, 128], F32, tag="mpt", name=f"pt_{mc}",
                                  bufs=2)
                    nc.tensor.transpose(pt[:], ofT[:, mc, :], ident[:])
                    nc.scalar.copy(objf[:, mc, :], pt[:])

            # ---- scatter groups (MLP pools closed; PSUM free) ----
            with (
                tc.tile_pool(name="scat", bufs=2) as scp,
                tc.tile_pool(name="cgps", bufs=4, space="PSUM") as cgps,
                tc.tile_pool(name="scps", bufs=4, space="PSUM") as scps,
            ):
                for gs0 in range(0, R_BUF, G_ROWS):
                    ge0 = min(gs0 + G_ROWS, R_BUF)
                    rows = ge0 - gs0
                    npx = rows * W
                    cov = scp.tile([32, G_ROWS, W], F16, tag="cov",
                                   name=f"cov_{gs0}")
                    nc.vector.tensor_tensor(
                        out=cov[:, :rows, :],
                        in0=covery[:, gs0:ge0].unsqueeze(2)
                            .broadcast_to([32, rows, W]),
                        in1=coverx[:].unsqueeze(1)
                            .broadcast_to([32, rows, W]),
                        op=ALU.mult)
                    cov2 = cov[:, :rows, :].rearrange("n r w -> n (r w)")
                    oh = scp.tile([32, G_ROWS, W], F16, tag="oh",
                                  name=f"oh_{gs0}")
                    oh2 = oh[:, :rows, :].rearrange("n r w -> n (r w)")
                    nsl = npx // 512
                    for sl in range(nsl):
                        cg = cgps.tile([32, 512], F32, tag="cg",
                                       name=f"cg_{gs0}_{sl}")
                        nc.tensor.matmul(cg[:], Ur[:], cov2[:, ts(sl, 512)],
                                         start=True, stop=True)
                        # onehot = (count_gt == 0) * cover
                        nc.vector.scalar_tensor_tensor(
                            out=oh2[:, ts(sl, 512)], in0=cg[:], scalar=0.0,
                            in1=cov2[:, ts(sl, 512)],
                            op0=ALU.is_equal, op1=ALU.mult)
                    for mc in range(2):
                        xsb = scp.tile([128, G_ROWS, W], F16, tag="xsb",
                                       name=f"xsb_{gs0}_{mc}")
                        x2 = xsb[:, :rows, :].rearrange("c r w -> c (r w)")
                        for sl in range(nsl):
                            sp = scps.tile([128, 512], F32, tag="sc",
                                           name=f"sp_{gs0}_{mc}_{sl}")
                            nc.tensor.matmul(sp[:], objf[:, mc, :],
                                             oh2[:, ts(sl, 512)],
                                             start=True, stop=True)
                            nc.scalar.copy(x2[:, ts(sl, 512)], sp[:])
                        nc.sync.dma_start(x0[mc, :, gs0:ge0, :],
                                          xsb[:, :rows, :])

            # ---- conv stages ----
            convs = [
                dict(g=0, src="raw", src_t=x0, out_t=y1_0, lo=1, hi=139),
                dict(g=1, src="bn", src_t=y1_0, sg=0, out_t=y2_0, lo=2, hi=138),
                dict(g=2, src="res", src_t=y2_0, res_t=x0, sg=1, out_t=y1_1,
                     r_out=r1, lo=3, hi=137),
                dict(g=3, src="bn", src_t=y1_1, sg=2, out_t=y2_1, lo=4, hi=136),
                dict(g=4, src="res", src_t=y2_1, res_t=r1, sg=3, out_t=y1_2,
                     r_out=r2, lo=5, hi=135),
                dict(g=5, src="bn", src_t=y1_2, sg=4, out_t=y2_2, lo=6, hi=134),
            ]

            with (
                tc.tile_pool(name="cw", bufs=2) as cwp,
                tc.tile_pool(name="cin", bufs=2) as cinp,
                tc.tile_pool(name="crt", bufs=3) as crtp,
                tc.tile_pool(name="cy", bufs=6) as cyp,
                tc.tile_pool(name="cst2", bufs=1) as cst2,
                tc.tile_pool(name="cps", bufs=8, space="PSUM") as cps,
            ):
                sqs = cst2.tile([128, 512], F32, name="sqs", bufs=2)
                for cv in convs:
                    g = cv["g"]
                    wr = cwp.tile([128, 18, C], F16, tag="wr", name=f"cwr_{g}")
                    nc.sync.dma_start(wr[:], convW_d[g])
                    st_sum = [cst2.tile([128, 64], F32, name=f"ssum_{g}_{m}",
                                        tag=f"ssum{m}") for m in range(2)]
                    st_sq = [cst2.tile([128, 64], F32, name=f"ssq_{g}_{m}",
                                       tag=f"ssq{m}") for m in range(2)]
                    glist = _groups(OWN0, OWN1)
                    # boundary groups last: their input halo row arrives via
                    # the cross-core row exchange of the previous stage
                    proc = glist[1:7] + [glist[0], glist[7]]
                    owned_idx = 0
                    for (start, end, owned) in proc:
                        cnt = end - start + 2
                        in_t = []
                        for kc in range(2):
                            it = cinp.tile([128, 18, W + 2], F16,
                                           tag=f"in{kc}",
                                           name=f"in_{g}_{start}_{kc}")
                            in_t.append(it)
                            sub = it[:, :cnt, 1:W + 1]
                            nc.sync.dma_start(
                                sub, cv["src_t"][kc, :, start - 1:end + 1, :])
                            # zero pad columns
                            nc.vector.tensor_copy(
                                it[:, :cnt, 0:1],
                                zeros[:].unsqueeze(1).broadcast_to([128, cnt, 1]))
                            nc.vector.tensor_copy(
                                it[:, :cnt, W + 1:W + 2],
                                zeros[:].unsqueeze(1).broadcast_to([128, cnt, 1]))
                            if cv["src"] == "bn":
                                col = kc * 6 + cv["sg"]
                                # chunked: PE starts on the first rows sooner
                                # at stage boundaries
                                for c0 in range(0, cnt, 6):
                                    c1 = min(c0 + 6, cnt)
                                    nc.scalar.activation(
                                        sub[:, c0:c1, :], sub[:, c0:c1, :],
                                        AF.Relu,
                                        bias=t_bn[:, col:col + 1],
                                        scale=s_bn[:, col:col + 1])
                            elif cv["src"] == "res":
                                col = kc * 6 + cv["sg"]
                                rt = crtp.tile([128, 18, W], F16, tag="rt",
                                               name=f"rt_{g}_{start}_{kc}")
                                nc.sync.dma_start(
                                    rt[:, :cnt, :],
                                    cv["res_t"][kc, :, start - 1:end + 1, :])
                                for c0 in range(0, cnt, 6):
                                    c1 = min(c0 + 6, cnt)
                                    nc.vector.scalar_tensor_tensor(
                                        out=sub[:, c0:c1, :],
                                        in0=sub[:, c0:c1, :],
                                        scalar=s_bn[:, col:col + 1],
                                        in1=rt[:, c0:c1, :],
                                        op0=ALU.mult, op1=ALU.add)
                                for c0 in range(0, cnt, 6):
                                    c1 = min(c0 + 6, cnt)
                                    nc.scalar.activation(
                                        sub[:, c0:c1, :], sub[:, c0:c1, :],
                                        AF.Relu,
                                        bias=t_bn[:, col:col + 1])
                            if cv["src"] != "raw":
                                # zero out image-invalid halo rows
                                if start < 7:
                                    k = min(7 - start, cnt)
                                    nc.vector.tensor_tensor(
                                        out=it[:, :k, 1:W + 1],
                                        in0=it[:, :k, 1:W + 1],
                                        in1=mask2d[:, start - 1:start - 1 + k]
                                            .unsqueeze(2)
                                            .broadcast_to([128, k, W]),
                                        op=ALU.mult)
                                if end > OWN1 - 1:
                                    k0 = (OWN1 - (start - 1))
                                    k = cnt - k0
                                    nc.vector.tensor_tensor(
                                        out=it[:, k0:cnt, 1:W + 1],
                                        in0=it[:, k0:cnt, 1:W + 1],
                                        in1=mask2d[:, start - 1 + k0:end + 1]
                                            .unsqueeze(2)
                                            .broadcast_to([128, k, W]),
                                        op=ALU.mult)
                            if cv["src"] == "res":
                                # write r_next back (skip 2-row overlap with
                                # the next group, except for the last group)
                                wlo, whi = start - 1, end + 1
                                if (start, end) != (glist[-1][0], glist[-1][1]):
                                    whi = end - 1
                                nc.sync.dma_start(
                                    cv["r_out"][kc, :, wlo:whi, :],
                                    it[:, :whi - wlo, 1:W + 1])
                        pairs = _rowpairs(start, end)
                        for mc in range(2):
                            psums = [cps.tile([128, 512], F32, tag="cp",
                                              name=f"ps_{g}_{start}_{mc}_{i}")
                                     for i in range(len(pairs))]
                            for t9 in range(9):
                                dy, dx = t9 // 3, t9 % 3
                                for kc in range(2):
                                    lhsT = wr[:, t9 * 2 + kc, ts(mc, 128)]
                                    for i, (pr, prn) in enumerate(pairs):
                                        loc = pr - (start - 1)
                                        rhs = in_t[kc][:, loc + dy - 1:
                                                       loc + dy - 1 + prn,
                                                       dx:dx + W]
                                        nc.tensor.matmul(
                                            psums[i][:, :prn * W], lhsT, rhs,
                                            start=(t9 == 0 and kc == 0),
                                            stop=(t9 == 8 and kc == 1))
                            for i, (pr, prn) in enumerate(pairs):
                                ysb = cyp.tile([128, 2, W], F16, tag="ysb",
                                               name=f"ysb_{g}_{start}_{mc}_{i}")
                                y2d = ysb[:, :prn, :].rearrange(
                                    "c r w -> c (r w)")
                                pv = psums[i][:, :prn * W]
                                if owned:
                                    idx = owned_idx + i
                                    nc.scalar.activation(
                                        y2d, pv, AF.Copy,
                                        accum_out=st_sum[mc][:, idx:idx + 1])
                                    nc.scalar.activation(
                                        sqs[:, :prn * W], pv, AF.Square,
                                        accum_out=st_sq[mc][:, idx:idx + 1])
                                else:
                                    nc.scalar.copy(y2d, pv)
                                nc.sync.dma_start(
                                    cv["out_t"][mc, :, pr:pr + prn, :],
                                    ysb[:, :prn, :])
                        if owned:
                            owned_idx += len(pairs)

                    # ---- BN stats: reduce, AllReduce pair, compute s/t ----
                    pay = cst2.tile([128, 4], F32, name=f"pay_{g}", tag="pay",
                                    bufs=2)
                    for m in range(2):
                        nc.vector.tensor_reduce(pay[:, 2 * m:2 * m + 1],
                                                st_sum[m][:],
                                                axis=mybir.AxisListType.X,
                                                op=ALU.add)
                        nc.vector.tensor_reduce(pay[:, 2 * m + 1:2 * m + 2],
                                                st_sq[m][:],
                                                axis=mybir.AxisListType.X,
                                                op=ALU.add)
                    sin = cst2.tile([128, 4], F32, name=f"sin_{g}", tag="sin",
                                    bufs=2)
                    cc_in = ccd.tile([128, 4], F32, name=f"ccin_{g}")
                    cc_out = ccd.tile([128, 4], F32, name=f"ccout_{g}")
                    # post the stats round-trip DMAs from the scalar queue
                    # (HWDGE): the SP queue is backlogged with feature-map
                    # traffic and would add ~10us to this critical chain
                    nc.scalar.dma_start(cc_in[:], pay[:])
                    nc.gpsimd.collective_compute(
                        "AllReduce", ALU.add,
                        replica_groups=[[0, 1], [2, 3], [4, 5], [6, 7]],
                        ins=[cc_in.opt()], outs=[cc_out.opt()])
                    nc.scalar.dma_start(sin[:], cc_out[:])
                    for m in range(2):
                        col = m * 6 + g
                        mean = cst2.tile([128, 1], F32, name=f"mean_{g}_{m}",
                                         tag="bnw0", bufs=2)
                        em2 = cst2.tile([128, 1], F32, name=f"em2_{g}_{m}",
                                        tag="bnw1", bufs=2)
                        nc.vector.tensor_scalar_mul(mean[:],
                                                    sin[:, 2 * m:2 * m + 1],
                                                    1.0 / NPIX_STATS)
                        nc.vector.tensor_scalar_mul(em2[:],
                                                    sin[:, 2 * m + 1:2 * m + 2],
                                                    1.0 / NPIX_STATS)
                        var = cst2.tile([128, 1], F32, name=f"var_{g}_{m}",
                                        tag="bnw2", bufs=2)
                        nc.vector.tensor_tensor(out=var[:], in0=mean[:],
                                                in1=mean[:], op=ALU.mult)
                        nc.vector.tensor_sub(var[:], em2[:], var[:])
                        sd = cst2.tile([128, 1], F32, name=f"sd_{g}_{m}",
                                       tag="bnw3", bufs=2)
                        nc.scalar.activation(sd[:], var[:], AF.Sqrt, bias=epsc[:])
                        inv = cst2.tile([128, 1], F32, name=f"inv_{g}_{m}",
                                        tag="bnw4", bufs=2)
                        nc.vector.reciprocal(inv[:], sd[:])
                        nc.vector.tensor_tensor(out=s_bn[:, col:col + 1],
                                                in0=gam[:, col:col + 1],
                                                in1=inv[:], op=ALU.mult)
                        tmp = cst2.tile([128, 1], F32, name=f"tmp_{g}_{m}",
                                        tag="bnw5", bufs=2)
                        nc.vector.tensor_tensor(out=tmp[:], in0=mean[:],
                                                in1=s_bn[:, col:col + 1],
                                                op=ALU.mult)
                        nc.vector.tensor_sub(t_bn[:, col:col + 1],
                                             bet[:, col:col + 1], tmp[:])

                    # ---- cross-core halo row exchange. Pairwise AllReduce
                    # (same proven path as the BN stats): each core
                    # contributes its first/last owned output row in its own
                    # slot (partner slot zeroed via the host-built rmask), so
                    # the sum is a concatenation. slot0 = even core's row 133
                    # (odd's buffer row 5); slot1 = odd core's row 6 (even's
                    # row 134). The image-invalid side of each pair is zeroed
                    # by mask2d at consumption.
                    if g < 5:
                        ot = cv["out_t"]
                        rbs = cst2.tile([128, 2, 2, W], F16, name=f"rbs_{g}",
                                        tag="rbs", bufs=2)
                        for kc in range(2):
                            nc.gpsimd.dma_start(rbs[:, kc, 0, :],
                                                ot[kc, :, OWN1 - 1, :])
                            nc.gpsimd.dma_start(rbs[:, kc, 1, :],
                                                ot[kc, :, OWN0, :])
                        nc.vector.tensor_tensor(out=rbs[:], in0=rbs[:],
                                                in1=rmask[:], op=ALU.mult)
                        cc_ri = ccd.tile([128, 2, 2, W], F16, name=f"ccri_{g}")
                        cc_ro = ccd.tile([128, 2, 2, W], F16, name=f"ccro_{g}")
                        nc.gpsimd.dma_start(cc_ri[:], rbs[:])
                        nc.gpsimd.collective_compute(
                            "AllReduce", ALU.add,
                            replica_groups=[[0, 1], [2, 3], [4, 5], [6, 7]],
                            ins=[cc_ri.opt()], outs=[cc_ro.opt()])
                        rbr = cst2.tile([128, 2, 2, W], F16, name=f"rbr_{g}",
                                        tag="rbr", bufs=2)
                        nc.gpsimd.dma_start(rbr[:], cc_ro[:])
                        for kc in range(2):
                            nc.gpsimd.dma_start(ot[kc, :, OWN0 - 1, :],
                                                rbr[:, kc, 0, :])
                            nc.gpsimd.dma_start(ot[kc, :, OWN1, :],
                                                rbr[:, kc, 1, :])

                # ---- final: out = relu(bn(y2_2) + r2), owned rows ----
                with tc.tile_pool(name="fin", bufs=2) as finp:
                    for i in range(128 // G_ROWS):
                        gs0 = OWN0 + i * G_ROWS
                        ge0 = gs0 + G_ROWS
                        for kc in range(2):
                            col = kc * 6 + 5
                            ft = finp.tile([128, G_ROWS, W], F16,
                                           tag=f"fy{kc}", name=f"ft_{i}_{kc}")
                            rt = finp.tile([128, G_ROWS, W], F16,
                                           tag=f"fr{kc}", name=f"frt_{i}_{kc}",
                                           bufs=3)
                            nc.sync.dma_start(ft[:], y2_2[kc, :, gs0:ge0, :])
                            nc.sync.dma_start(rt[:], r2[kc, :, gs0:ge0, :])
                            for c0 in range(0, G_ROWS, 8):
                                nc.vector.scalar_tensor_tensor(
                                    out=ft[:, c0:c0 + 8, :],
                                    in0=ft[:, c0:c0 + 8, :],
                                    scalar=s_bn[:, col:col + 1],
                                    in1=rt[:, c0:c0 + 8, :],
                                    op0=ALU.mult, op1=ALU.add)
                            for q in range(0, G_ROWS, 4):
                                osb = finp.tile([128, 4, W], F16, tag="osb",
                                                name=f"osb_{i}_{kc}_{q}",
                                                bufs=4)
                                nc.scalar.activation(osb[:], ft[:, q:q + 4, :],
                                                     AF.Relu,
                                                     bias=t_bn[:, col:col + 1])
                                nc.sync.dma_start(
                                    out_d[kc, :, gs0 - OWN0 + q:
                                          gs0 - OWN0 + q + 4, :],
                                    osb[:])

    nc.compile()
    return nc


def _host_inputs(pred_box, pred_score, W1, b1, W2, b2, W3, b3, conv_w,
                 gamma, beta):
    """Build the 8 per-core input maps."""
    f = np.float32
    # conv weights: [blk, j, co, ci, ky, kx] -> [g, ci128, (ky kx kc), co]
    cw = conv_w.reshape(6, 256, 2, 128, 3, 3)
    cw = np.ascontiguousarray(cw.transpose(0, 3, 4, 5, 2, 1))
    convW = cw.reshape(6, 128, 18, 256).astype(FH)
    gamT = np.ascontiguousarray(
        gamma.reshape(6, 2, 128).transpose(1, 2, 0)).reshape(2, 128, 6)
    betT = np.ascontiguousarray(
        beta.reshape(6, 2, 128).transpose(1, 2, 0)).reshape(2, 128, 6)
    gamT = np.concatenate([gamT[0], gamT[1]], axis=1)  # [128, 12]
    betT = np.concatenate([betT[0], betT[1]], axis=1)
    W2c = np.ascontiguousarray(W2.reshape(2, 128, 256).transpose(1, 0, 2))
    W3c = np.ascontiguousarray(W3.reshape(2, 128, 256).transpose(1, 0, 2))
    iota256 = np.broadcast_to(np.arange(W, dtype=f), (128, W)).copy()
    # count_gt[n] = sum_m lhsT[m, n] * cover[m]; lhsT[m, n] = 1 iff m > n
    U = (np.arange(N)[:, None] > np.arange(N)[None, :]).astype(FH)
    ones1 = np.ones((1, 128), f)

    in_maps = []
    for c in range(N_CORES):
        b = c // 2
        s = 128 * (c % 2)
        geom = pred_box[b].reshape(N, 24).astype(f)
        objT = np.concatenate([geom.T, pred_score[b][None, :].astype(f)], 0)
        cx = pred_box[b, :, :, 0].astype(f)
        cy = pred_box[b, :, :, 1].astype(f)
        gx = np.floor((cx + f(51.2)) / f(0.4))
        gy = np.floor((cy + f(51.2)) / f(0.4))
        gx0 = np.clip(gx.min(-1), 0, W - 1).astype(f)
        gx1v = np.clip(gx.max(-1), 0, W - 1).astype(f)
        gy0 = np.clip(gy.min(-1), 0, H - 1).astype(f)
        gy1v = np.clip(gy.max(-1), 0, H - 1).astype(f)
        rows = np.arange(s - HALO, s + 128 + HALO, dtype=np.int64)
        iotaH = np.broadcast_to(rows.astype(f), (128, R_BUF)).copy()
        mask2d = np.broadcast_to(
            ((rows >= 0) & (rows < H)).astype(FH), (128, R_BUF)).copy()
        rmaskx = np.zeros((128, 2, 2, W), FH)
        rmaskx[:, :, c % 2, :] = 1
        in_maps.append({
            "objT": np.ascontiguousarray(objT),
            "rmaskx": rmaskx,
            "gx0m": (gx0 - 1)[:, None], "gx1": gx1v[:, None],
            "gy0m": (gy0 - 1)[:, None], "gy1": gy1v[:, None],
            "score_row": pred_score[b][None, :].astype(f),
            "W1": W1.astype(f), "W2c": W2c.astype(f), "W3c": W3c.astype(f),
            "b1c": b1.reshape(2, 128).T.astype(f).copy(),
            "b2c": b2.reshape(2, 128).T.astype(f).copy(),
            "b3c": b3.reshape(2, 128).T.astype(f).copy(),
            "convW": convW,
            "gammaT": gamT.astype(f), "betaT": betT.astype(f),
            "iota256": iota256, "iotaH": iotaH, "mask2d": mask2d,
            "onesK1": ones1, "Utri": U,
        })
    return in_maps


def kernel(**inputs):
    global _PROGRAM, LAST_EXEC_NS
    if _PROGRAM is None:
        _PROGRAM = _build_program()
    nc = _PROGRAM
    in_maps = _host_inputs(**{k: np.asarray(v) for k, v in inputs.items()})
    res = bass_utils.run_bass_kernel_spmd(
        nc, in_maps, core_ids=list(range(N_CORES)), trace=_TRACE)
    LAST_EXEC_NS = res.exec_time_ns
    full = np.empty((B, C, H, W), np.float32)
    for c in range(N_CORES):
        b = c // 2
        s = 128 * (c % 2)
        o = res.results[c]["out"]
        full[b, 0:128, s:s + 128, :] = o[0]
        full[b, 128:256, s:s + 128, :] = o[1]
    return full
